# revision 43
# baseline (speedup 1.0000x reference)
"""TRN2 Bass kernel for nn_MoEPositionwiseFFN: kernel(**inputs) -> np.ndarray.

v2: expert-parallel MoE FFN without dynamic-DMA dispatch scatter.

Per core r (= expert r):
  P1  router on own 2048 tokens (fp32), AllGather 4 routing planes.
  P2  replicated capacity selection (threshold bisection) -> keep/pos.
  P2.5 own-token extraction; slot->token dispatch table built with
       one-hot rank-1 matmuls into PSUM; ReduceScatter(add) routes each
       expert its [CAPP] slice. Combine indices built via PE transposes.
  P3  expert FFN on 2560 gathered rows (row dma_gather + PE transpose),
       output AllGather chunked (5x) to overlap with compute.
  P4  combine: gather 2 rows/token from all_out, gate-weighted sum
       (gates applied combine-side; ZSLOT row zeroed explicitly).
"""

import os
import sys

for _p in ("/opt/trn_rl_repo", "/opt/pypackages"):
    if _p not in sys.path:
        sys.path.insert(0, _p)


from dataclasses import dataclass

import numpy as np

import concourse.bass as bass
import concourse.bacc as bacc
import concourse.tile as tile
import concourse.mybir as mybir
from concourse.masks import make_identity

FP32 = mybir.dt.float32
BF16 = mybir.dt.bfloat16
I32 = mybir.dt.int32
I16 = mybir.dt.int16
U16 = mybir.dt.uint16
AF = mybir.ActivationFunctionType
ALU = mybir.AluOpType
AX = mybir.AxisListType


@dataclass
class Cfg:
    ncores: int = 8
    E: int = 8
    K: int = 2
    D: int = 1024
    H: int = 4096
    TPC: int = 2048          # tokens per core
    cap: int = 2458          # reference capacity
    CAPP: int = 2560         # padded capacity (= NCHUNK*CB)
    CB: int = 512            # FFN chunk / AllGather chunk (tokens)
    NBIS: int = 26           # bisection iterations (covers 0x3C000000..0x3F800000)

    @property
    def N(self):
        return self.ncores * self.TPC

    @property
    def TT(self):
        return self.TPC // 128  # token tiles per core (16)

    @property
    def M(self):
        return self.N // 128    # global token groups (128)

    @property
    def DCH(self):
        return self.D // 128

    @property
    def HCH(self):
        return self.H // 128

    @property
    def NCHUNK(self):
        return self.CAPP // self.CB

    @property
    def FW(self):
        return self.CAPP // 16  # dispatch-table free width (160)


def build(cfg: Cfg, dbg: bool = False):
    E, K, D, H = cfg.E, cfg.K, cfg.D, cfg.H
    TPC, TT, M, N = cfg.TPC, cfg.TT, cfg.M, cfg.N
    DCH, HCH = cfg.DCH, cfg.HCH
    CAP, CAPP, CB, NCHUNK, FW = cfg.cap, cfg.CAPP, cfg.CB, cfg.NCHUNK, cfg.FW
    NC = cfg.ncores
    assert E == NC == 8 and K == 2
    assert CAPP == NCHUNK * CB and CAPP % 16 == 0 and CB % 128 == 0
    NBLK = CB // 128          # 128-token blocks per chunk (4)
    # combine flat row for dropped assignments: chunk layout row of
    # (expert 0, slot CAPP-1), guaranteed zeroed.
    ZC = (CAPP - 1) // CB
    ZOFF = ZC * (NC * CB) + 0 * CB + ((CAPP - 1) - ZC * CB)

    nc = bacc.Bacc("TRN2", target_bir_lowering=False, debug=False,
                   num_devices=NC)

    # ---- external inputs (per-core staged by host) ----
    xT_shard = nc.dram_tensor("xT_shard", [D, TPC], FP32, kind="ExternalInput")
    x_bf16 = nc.dram_tensor("x_bf16", [N, D], BF16, kind="ExternalInput")
    Wr_in = nc.dram_tensor("Wr_in", [128, DCH, E], FP32, kind="ExternalInput")
    br_in = nc.dram_tensor("br_in", [1, E], FP32, kind="ExternalInput")
    W1_in = nc.dram_tensor("W1_in", [128, DCH, H], BF16, kind="ExternalInput")
    W2_in = nc.dram_tensor("W2_in", [128, HCH, D], BF16, kind="ExternalInput")
    b1_in = nc.dram_tensor("b1_in", [128, HCH], FP32, kind="ExternalInput")
    b2_in = nc.dram_tensor("b2_in", [1, D], BF16, kind="ExternalInput")
    ltri_in = nc.dram_tensor("ltri_in", [128, 128], BF16, kind="ExternalInput")
    rk_in = nc.dram_tensor("rk_in", [128, 1], FP32, kind="ExternalInput")

    # ---- external output ----
    y_out = nc.dram_tensor("y_out", [TPC, D], FP32, kind="ExternalOutput")

    if dbg:
        dbg_rta = nc.dram_tensor("dbg_rta", [4, 128, M], FP32,
                                 kind="ExternalOutput")
        dbg_pos = nc.dram_tensor("dbg_pos", [128, E * M], FP32,
                                 kind="ExternalOutput")
        dbg_keep = nc.dram_tensor("dbg_keep", [128, E * M], FP32,
                                  kind="ExternalOutput")
        dbg_tab = nc.dram_tensor("dbg_tab", [16, FW], FP32,
                                 kind="ExternalOutput")
        dbg_cidx = nc.dram_tensor("dbg_cidx", [128, K * TT * 8], I16,
                                  kind="ExternalOutput")
        dbg_didx = nc.dram_tensor("dbg_didx", [128, FW], I16,
                                  kind="ExternalOutput")
        dbg_oe = nc.dram_tensor("dbg_oe", [CAPP, D], BF16,
                                kind="ExternalOutput")

    with tile.TileContext(nc) as tc:
        rank_sp = nc.partition_id()

        import contextlib
        top = contextlib.ExitStack()
        cpool = top.enter_context(tc.tile_pool(name="const", bufs=1))
        wts = top.enter_context(tc.tile_pool(name="wts", bufs=1))
        keepp = top.enter_context(tc.tile_pool(name="keepp", bufs=1))
        dramp = top.enter_context(tc.tile_pool(name="dramp", bufs=1,
                                               space="DRAM"))

        # ---- DRAM scratch ----
        rt_local = dramp.tile([4, 128, TT], FP32, tag="rt_local")
        rt_all = dramp.tile([NC, 4, 128, TT], FP32, tag="rt_all",
                            addr_space="Shared")
        tab_dram = dramp.tile([128, FW], FP32, tag="tab_dram")
        tab_all = dramp.tile([NC, 128, FW], FP32, tag="tab_all",
                             addr_space="Shared")
        out_e = dramp.tile([CAPP, D], BF16, tag="out_e")
        AG_CHUNKED = True
        if AG_CHUNKED:
            all_out = dramp.tile([NCHUNK, NC, CB, D], BF16, tag="all_out")
            ag_c = []
            for c in range(NCHUNK):
                agt = dramp.tile([NC, CB, D], BF16, tag=f"agc{c}",
                                 addr_space="Shared")
                ag_c.append(agt)
            # finer-grained AG for the last chunk (halves tail exposure)
            ag_h = []
            for h in range(2):
                agh = dramp.tile([NC, CB // 2, D], BF16, tag=f"agh{h}",
                                 addr_space="Shared")
                ag_h.append(agh)
        else:
            all_out = dramp.tile([NC, CAPP, D], BF16, tag="all_out",
                                 addr_space="Shared")

        # ---- weights: DMA starts immediately, overlaps P1/P2 ----
        W1s = wts.tile([128, DCH, H], BF16, tag="W1s")
        nc.sync.dma_start(W1s, W1_in[:, :, :])
        W2s = wts.tile([128, HCH, D], BF16, tag="W2s")
        nc.sync.dma_start(W2s, W2_in[:, :, :])
        b1s = wts.tile([128, HCH], FP32, tag="b1s")
        nc.sync.dma_start(b1s, b1_in[:, :])
        b2s = wts.tile([1, D], BF16, tag="b2s")
        nc.sync.dma_start(b2s, b2_in[:, :])

        # ---- constants ----
        ident_b = cpool.tile([128, 128], BF16, tag="ident_b")
        make_identity(nc, ident_b)
        ident_f = cpool.tile([128, 128], FP32, tag="ident_f")
        make_identity(nc, ident_f)
        ltri = cpool.tile([128, 128], BF16, tag="ltri")
        nc.sync.dma_start(ltri, ltri_in[:, :])
        wr_sb = cpool.tile([128, DCH, E], FP32, tag="wr")
        nc.sync.dma_start(wr_sb, Wr_in[:, :, :])
        br_sb = cpool.tile([1, E], FP32, tag="br")
        nc.sync.dma_start(br_sb, br_in[:, :])
        rk_sb = cpool.tile([128, 1], FP32, tag="rk_sb")
        nc.sync.dma_start(rk_sb, rk_in[:, :])
        ones1f = cpool.tile([1, 128], FP32, tag="ones1f")
        nc.vector.memset(ones1f, 1.0)
        ones1b = cpool.tile([1, 128], BF16, tag="ones1b")
        nc.vector.memset(ones1b, 1.0)
        ones128b = cpool.tile([128, 128], BF16, tag="ones128b")
        nc.vector.memset(ones128b, 1.0)
        one_i = cpool.tile([128, E], I32, tag="one_i")
        nc.vector.memset(one_i, 1)
        # iota along free: F160[p, j] = j ; F128 = F160[:, :128]
        it_i = cpool.tile([128, FW], I32, tag="it_i")
        nc.gpsimd.iota(it_i, pattern=[[1, FW]], base=0, channel_multiplier=0)
        F160 = cpool.tile([128, FW], FP32, tag="F160")
        nc.vector.tensor_copy(F160, it_i)
        F128 = F160[:, 0:128]
        # lovals[p, 0] = p
        lov_i = cpool.tile([128, 1], I32, tag="lov_i")
        nc.gpsimd.iota(lov_i, pattern=[[1, 1]], base=0, channel_multiplier=1)
        lovals = cpool.tile([128, 1], FP32, tag="lovals")
        nc.vector.tensor_copy(lovals, lov_i)
        # hival[p, t] = rank*TT + t   (token-group id of own tile t)
        tt_i = cpool.tile([128, TT], I32, tag="tt_i")
        nc.gpsimd.iota(tt_i, pattern=[[1, TT]], base=0, channel_multiplier=0)
        hival = cpool.tile([128, TT], FP32, tag="hival")
        nc.vector.tensor_copy(hival, tt_i)
        nc.vector.tensor_scalar(hival, hival, rk_sb, None, op0=ALU.add)
        # zmask[p] = 0 for p == 127 else 1 (ZSLOT row kill)
        zmask = cpool.tile([128, 1], FP32, tag="zmask")
        nc.vector.tensor_scalar(zmask, lovals, 127.0, None,
                                op0=ALU.not_equal)
        # tokv[p, t] = global token id of own (t, p) = (rk+t)*128 + p
        tokv = cpool.tile([128, TT], FP32, tag="tokv")
        nc.vector.tensor_scalar(tokv, hival, 128.0, lovals,
                                op0=ALU.mult, op1=ALU.add)

        # ---- persistent small tiles (survive into P3/P4) ----
        dIdx = keepp.tile([128, FW], I16, tag="dIdx")
        ci16 = keepp.tile([128, K * TT * 8], I16, tag="ci16")
        g1o = keepp.tile([128, TT], FP32, tag="g1o")
        g2o = keepp.tile([128, TT], FP32, tag="g2o")

        selstack = contextlib.ExitStack()
        sel = selstack.enter_context(tc.tile_pool(name="sel", bufs=1))
        xpool = selstack.enter_context(tc.tile_pool(name="xpool", bufs=2))
        lrp = selstack.enter_context(tc.tile_pool(name="lrp", bufs=2))
        psr = selstack.enter_context(
            tc.tile_pool(name="psr", bufs=2, space="PSUM"))
        pscnt = selstack.enter_context(
            tc.tile_pool(name="pscnt", bufs=2, space="PSUM"))
        pstab = selstack.enter_context(
            tc.tile_pool(name="pstab", bufs=1, space="PSUM"))
        pstr = selstack.enter_context(
            tc.tile_pool(name="pstr", bufs=1, space="PSUM"))

        # ---------- P1: router on own shard ----------
        E_sb = sel.tile([128, TT, E], FP32, tag="E_sb")
        QT = TT // 4  # t-tiles per quarter
        for q4 in range(4):
            xq = xpool.tile([128, DCH, QT * 128], FP32, tag="xq")
            nc.sync.dma_start(
                xq,
                xT_shard[:, q4 * QT * 128:(q4 + 1) * QT * 128]
                .rearrange("(dch p) t -> p dch t", p=128))
            for tr in range(QT):
                t = q4 * QT + tr
                ps = psr.tile([128, E], FP32, tag="psr")
                for dch in range(DCH):
                    nc.tensor.matmul(
                        ps, lhsT=xq[:, dch, tr * 128:(tr + 1) * 128],
                        rhs=wr_sb[:, dch, :],
                        start=(dch == 0), stop=False)
                nc.tensor.matmul(ps, lhsT=ones1f, rhs=br_sb[:, :],
                                 start=False, stop=True)
                nc.scalar.activation(E_sb[:, t, :], ps, AF.Exp)
        # batched top-2 over the expert axis
        e8i = sel.tile([128, TT * 8], I16, tag="e8i")
        nc.gpsimd.iota(e8i, pattern=[[0, TT], [1, 8]], base=0,
                       channel_multiplier=0)
        e8f = sel.tile([128, TT, 8], FP32, tag="e8f")
        nc.vector.tensor_copy(e8f.rearrange("p a b -> p (a b)"), e8i)
        Z_sb = sel.tile([128, TT], FP32, tag="Z_sb")
        nc.vector.tensor_reduce(Z_sb, E_sb, AX.X, ALU.add)
        m1 = sel.tile([128, TT], FP32, tag="m1")
        nc.vector.tensor_reduce(m1, E_sb, AX.X, ALU.max)
        eqx = sel.tile([128, TT, 8], FP32, tag="eqx")
        nc.vector.tensor_tensor(
            eqx, E_sb, m1.unsqueeze(2).broadcast_to((128, TT, 8)),
            ALU.is_equal)
        tmp8 = sel.tile([128, TT, 8], FP32, tag="tmp8")
        nc.vector.tensor_tensor(tmp8, eqx, e8f, ALU.mult)
        P_i1 = sel.tile([128, TT], FP32, tag="P_i1")
        nc.vector.tensor_reduce(P_i1, tmp8, AX.X, ALU.add)
        nc.vector.tensor_scalar(tmp8, eqx, -1e30, None, op0=ALU.mult)
        nc.vector.tensor_tensor(E_sb, E_sb, tmp8, ALU.add)  # mask out top-1
        m2 = sel.tile([128, TT], FP32, tag="m2")
        nc.vector.tensor_reduce(m2, E_sb, AX.X, ALU.max)
        nc.vector.tensor_tensor(
            eqx, E_sb, m2.unsqueeze(2).broadcast_to((128, TT, 8)),
            ALU.is_equal)
        nc.vector.tensor_tensor(tmp8, eqx, e8f, ALU.mult)
        P_i2 = sel.tile([128, TT], FP32, tag="P_i2")
        nc.vector.tensor_reduce(P_i2, tmp8, AX.X, ALU.add)
        rZ = sel.tile([128, TT], FP32, tag="rZ")
        nc.vector.reciprocal(rZ, Z_sb)
        P_g1 = sel.tile([128, TT], FP32, tag="P_g1")
        P_g2 = sel.tile([128, TT], FP32, tag="P_g2")
        nc.vector.tensor_tensor(P_g1, m1, rZ, ALU.mult)
        nc.vector.tensor_tensor(P_g2, m2, rZ, ALU.mult)
        nc.sync.dma_start(rt_local[0], P_i1)
        nc.sync.dma_start(rt_local[1], P_g1)
        nc.sync.dma_start(rt_local[2], P_i2)
        nc.sync.dma_start(rt_local[3], P_g2)
        nc.gpsimd.collective_compute(
            "AllGather", ALU.bypass,
            replica_groups=[list(range(NC))],
            ins=[rt_local.opt()], outs=[rt_all.opt()])

        # ---------- P2: replicated selection ----------
        i1f = sel.tile([128, M], FP32, tag="i1f")
        g1f = sel.tile([128, M], FP32, tag="g1f")
        i2f = sel.tile([128, M], FP32, tag="i2f")
        g2f = sel.tile([128, M], FP32, tag="g2f")
        for q, dst in ((0, i1f), (1, g1f), (2, i2f), (3, g2f)):
            nc.sync.dma_start(
                dst.rearrange("p (r t) -> p r t", r=NC),
                rt_all[:, q, :, :].rearrange("r p t -> p r t"))
        if dbg:
            nc.sync.dma_start(dbg_rta[0], i1f)
            nc.sync.dma_start(dbg_rta[1], g1f)
            nc.sync.dma_start(dbg_rta[2], i2f)
            nc.sync.dma_start(dbg_rta[3], g2f)

        A_sb = sel.tile([128, E, M], FP32, tag="A_sb")
        tmpM = sel.tile([128, M], FP32, tag="tmpM")
        for e in range(E):
            nc.vector.scalar_tensor_tensor(
                A_sb[:, e, :], i1f, float(e), g1f,
                op0=ALU.is_equal, op1=ALU.mult)
            nc.vector.scalar_tensor_tensor(
                tmpM, i2f, float(e), g2f, op0=ALU.is_equal, op1=ALU.mult)
            nc.vector.tensor_tensor(A_sb[:, e, :], A_sb[:, e, :], tmpM,
                                    ALU.add)

        big = sel.tile([128, E, M], FP32, tag="big")
        cntp = sel.tile([128, E], FP32, tag="cntp")
        cntb = sel.tile([128, E], BF16, tag="cntb")
        cntf = sel.tile([128, E], FP32, tag="cntf")
        Ktgt = sel.tile([128, E], FP32, tag="Ktgt")
        lo = sel.tile([128, E], I32, tag="lo")
        hi = sel.tile([128, E], I32, tag="hi")
        mid = sel.tile([128, E], I32, tag="mid")
        condi = sel.tile([128, E], I32, tag="condi")

        nc.vector.tensor_scalar(big, A_sb, 0.0, None, op0=ALU.is_gt)
        nc.vector.tensor_reduce(cntp, big, AX.X, ALU.add)
        nc.vector.tensor_copy(cntb, cntp)
        pc = pscnt.tile([128, E], FP32, tag="pscnt")
        nc.tensor.matmul(pc, lhsT=ones128b, rhs=cntb, start=True, stop=True)
        nc.vector.tensor_scalar(Ktgt, pc, float(CAP), None, op0=ALU.min)

        zerosM = sel.tile([128, M], FP32, tag="zerosM")
        nc.vector.memset(zerosM, 0.0)
        nc.vector.memset(lo, 0x3C000000)
        nc.vector.memset(hi, 0x3F800000)
        for it in range(cfg.NBIS):
            nc.vector.tensor_tensor(mid, lo, hi, ALU.add)
            nc.vector.tensor_tensor(mid, mid, one_i,
                                    ALU.logical_shift_right)
            midf = mid.bitcast(FP32)
            for e in range(E):
                nc.vector.scalar_tensor_tensor(
                    big[:, e, :], A_sb[:, e, :], midf[:, e:e + 1], zerosM,
                    op0=ALU.is_gt, op1=ALU.add,
                    accum_out=cntp[:, e:e + 1])
            nc.vector.tensor_copy(cntb, cntp)
            pc = pscnt.tile([128, E], FP32, tag="pscnt")
            nc.tensor.matmul(pc, lhsT=ones128b, rhs=cntb, start=True,
                             stop=True)
            nc.vector.tensor_copy(cntf, pc)
            nc.vector.tensor_tensor(condi, cntf, Ktgt, ALU.is_ge)
            nc.vector.copy_predicated(lo, condi, mid)
            nc.vector.tensor_tensor(condi, cntf, Ktgt, ALU.is_lt)
            nc.vector.copy_predicated(hi, condi, mid)

        thrf = lo.bitcast(FP32)
        keepf = sel.tile([128, E, M], FP32, tag="keepf")
        nc.vector.tensor_tensor(
            keepf, A_sb, thrf.unsqueeze(2).broadcast_to((128, E, M)),
            ALU.is_gt)

        rp = sel.tile([128, E, M], FP32, tag="rp")
        for e in range(E):
            nc.vector.tensor_tensor_scan(
                rp[:, e, :], keepf[:, e, :], zerosM, initial=0.0,
                op0=ALU.add, op1=ALU.add)
        totb = sel.tile([128, E], BF16, tag="totb")
        nc.vector.tensor_copy(totb, rp[:, :, M - 1])
        pe_x = pscnt.tile([128, E], FP32, tag="pscnt")
        nc.tensor.matmul(pe_x, lhsT=ltri, rhs=totb, start=True, stop=True)
        excl = sel.tile([128, E], FP32, tag="excl")
        nc.vector.tensor_copy(excl, pe_x)
        pos = sel.tile([128, E, M], FP32, tag="pos")
        nc.vector.tensor_tensor(pos, rp, keepf, ALU.subtract)
        nc.vector.tensor_tensor(
            pos, pos, excl.unsqueeze(2).broadcast_to((128, E, M)),
            ALU.add)
        if dbg:
            nc.sync.dma_start(dbg_pos[:, :],
                              pos.rearrange("p e m -> p (e m)"))
            nc.sync.dma_start(dbg_keep[:, :],
                              keepf.rearrange("p e m -> p (e m)"))

        # ---------- P2.5: own-token extraction + dispatch table ----------
        own0 = bass.ds(rank_sp * TT, TT)
        c15 = sel.tile([128, TT], I32, tag="c15")
        nc.vector.memset(c15, 15)
        c4 = sel.tile([128, TT], I32, tag="c4")
        nc.vector.memset(c4, 4)
        c511 = sel.tile([128, TT], I32, tag="c511")
        nc.vector.memset(c511, 511)
        c9 = sel.tile([128, TT], I32, tag="c9")
        nc.vector.memset(c9, 9)
        pl_k = []      # [128, TT] fp32 per k: table row  e*16 + pos%16
        fs_k = []      # [128, TT] fp32 per k: table col  pos//16 (999=dead)
        offall = sel.tile([128, K * TT], FP32, tag="offall")
        tmpT = sel.tile([128, TT], FP32, tag="tmpT")
        for k in range(K):
            ikf = i1f if k == 0 else i2f
            gkf = g1f if k == 0 else g2f
            go = g1o if k == 0 else g2o
            nc.vector.tensor_copy(go, gkf[:, own0])
            iko = sel.tile([128, TT], FP32, tag=f"iko{k}")
            nc.vector.tensor_copy(iko, ikf[:, own0])
            posk = sel.tile([128, TT], FP32, tag=f"posk{k}")
            keepk = sel.tile([128, TT], FP32, tag=f"keepk{k}")
            first = True
            for e in range(E):
                dst = posk if first else tmpT
                nc.vector.scalar_tensor_tensor(
                    dst, iko, float(e), pos[:, e, own0],
                    op0=ALU.is_equal, op1=ALU.mult)
                if not first:
                    nc.vector.tensor_tensor(posk, posk, tmpT, ALU.add)
                first = False
            first = True
            for e in range(E):
                dst = keepk if first else tmpT
                nc.vector.scalar_tensor_tensor(
                    dst, iko, float(e), keepf[:, e, own0],
                    op0=ALU.is_equal, op1=ALU.mult)
                if not first:
                    nc.vector.tensor_tensor(keepk, keepk, tmpT, ALU.add)
                first = False
            keepi = sel.tile([128, TT], I32, tag=f"keepi{k}")
            nc.vector.tensor_copy(keepi, keepk)
            # table coords (integer split of pos: %16 and //16)
            posI = sel.tile([128, TT], I32, tag=f"posI{k}")
            nc.vector.tensor_copy(posI, posk)
            tmpI = sel.tile([128, TT], I32, tag=f"tmpI{k}")
            nc.vector.tensor_tensor(tmpI, posI, c15, ALU.bitwise_and)
            qo = sel.tile([128, TT], FP32, tag=f"qo{k}")
            nc.vector.tensor_copy(qo, tmpI)
            nc.vector.tensor_tensor(tmpI, posI, c4, ALU.logical_shift_right)
            fo = sel.tile([128, TT], FP32, tag=f"fo{k}")
            nc.vector.tensor_copy(fo, tmpI)
            plo = sel.tile([128, TT], FP32, tag=f"plo{k}")
            nc.vector.scalar_tensor_tensor(
                plo, iko, 16.0, qo, op0=ALU.mult, op1=ALU.add)
            fsel = sel.tile([128, TT], FP32, tag=f"fsel{k}")
            nc.vector.memset(fsel, 999.0)
            nc.vector.copy_predicated(fsel, keepi, fo)
            pl_k.append(plo)
            fs_k.append(fsel)
            off = sel.tile([128, TT], FP32, tag=f"off{k}")
            if AG_CHUNKED:
                # combine flat row (chunk layout): c = pos//CB;
                # off = c*(NC*CB) + ik*CB + pos%CB ; dropped -> ZOFF
                nc.vector.tensor_tensor(tmpI, posI, c511, ALU.bitwise_and)
                m5 = sel.tile([128, TT], FP32, tag=f"m5{k}")
                nc.vector.tensor_copy(m5, tmpI)
                nc.vector.tensor_tensor(tmpI, posI, c9,
                                        ALU.logical_shift_right)
                cdv = sel.tile([128, TT], FP32, tag=f"cdv{k}")
                nc.vector.tensor_copy(cdv, tmpI)
                nc.vector.tensor_scalar(cdv, cdv, float(NC * CB), None,
                                        op0=ALU.mult)
                nc.vector.scalar_tensor_tensor(
                    off, iko, float(CB), m5, op0=ALU.mult, op1=ALU.add)
                nc.vector.tensor_tensor(off, off, cdv, ALU.add)
                zoff = float(ZOFF)
            else:
                # off = ik*CAPP + pos ; dropped -> expert0 slot CAPP-1
                nc.vector.scalar_tensor_tensor(
                    off, iko, float(CAPP), posk, op0=ALU.mult, op1=ALU.add)
                zoff = float(CAPP - 1)
            offd = sel.tile([128, TT], FP32, tag=f"offd{k}")
            nc.vector.memset(offd, zoff)
            nc.vector.copy_predicated(offd, keepi, off)
            nc.vector.tensor_copy(offall[:, k * TT:(k + 1) * TT], offd)

        # table build: psTab[row, :] += sum over items of onehot outer
        # (single fp32 plane carrying the token id directly)
        psTab = pstab.tile([128, FW], FP32, tag="psTab")
        nck = 0
        for k in range(K):
            for t in range(TT):
                L = lrp.tile([128, 128], FP32, tag="L")
                nc.vector.tensor_scalar(
                    L, F128, pl_k[k][:, t:t + 1], None, op0=ALU.is_equal)
                R = lrp.tile([128, FW], FP32, tag="R")
                nc.vector.tensor_scalar(
                    R, F160, fs_k[k][:, t:t + 1], tokv[:, t:t + 1],
                    op0=ALU.is_equal, op1=ALU.mult)
                nc.tensor.matmul(psTab, lhsT=L, rhs=R,
                                 start=(nck == 0), stop=(nck == K * TT - 1))
                nck += 1
        tabsb = sel.tile([128, FW], FP32, tag="tabsb")
        nc.vector.tensor_copy(tabsb, psTab)
        nc.sync.dma_start(tab_dram, tabsb)
        nc.gpsimd.collective_compute(
            "AllGather", ALU.bypass,
            replica_groups=[list(range(NC))],
            ins=[tab_dram.opt()], outs=[tab_all.opt()])

        # readback own expert's 16 rows from each core's table and sum
        own16 = bass.ds(rank_sp * 16, 16)
        tabs = sel.tile([16, FW], FP32, tag="tabs")
        tabr = sel.tile([16, NC, FW], FP32, tag="tabr")
        for r in range(NC):
            nc.sync.dma_start(tabr[:, r, :], tab_all[r, own16, :])
        nc.vector.tensor_tensor(tabs, tabr[:, 0, :], tabr[:, 1, :], ALU.add)
        for r in range(2, NC):
            nc.vector.tensor_tensor(tabs, tabs, tabr[:, r, :], ALU.add)
        if dbg:
            nc.sync.dma_start(dbg_tab, tabs)
        dI16 = sel.tile([16, FW], I16, tag="dI16")
        nc.vector.tensor_copy(dI16, tabs)
        for g in range(8):
            nc.sync.dma_start(dIdx[g * 16:(g + 1) * 16, :], dI16)

        # combine idx relayout via PE transposes:
        # cidxf[pl, (k t), ph] = offall[ph*16+pl, (k t)]
        psO = pstr.tile([32, 128], FP32, tag="psO")
        nc.tensor.transpose(psO, offall, ident_f)
        T1 = sel.tile([32, 128], FP32, tag="T1")
        nc.vector.tensor_copy(T1, psO)
        cidxf = sel.tile([128, K * TT, 8], FP32, tag="cidxf")
        for ph in range(8):
            psP = pstr.tile([16, 32], FP32, tag="psP")
            nc.tensor.transpose(psP, T1[:, ph * 16:(ph + 1) * 16],
                                ident_f[0:32, 0:32])
            nc.vector.tensor_copy(cidxf[0:16, :, ph], psP)
        cs16 = sel.tile([16, K * TT * 8], I16, tag="cs16")
        nc.vector.tensor_copy(
            cs16, cidxf[0:16].rearrange("p a b -> p (a b)"))
        for g in range(8):
            nc.sync.dma_start(ci16[g * 16:(g + 1) * 16, :], cs16)
        if dbg:
            nc.sync.dma_start(dbg_cidx, ci16)
            nc.sync.dma_start(dbg_didx, dIdx)

        selstack.close()

        # ---------- P3: expert FFN ----------
        with tc.tile_pool(name="ffn", bufs=2) as ffn, \
             tc.tile_pool(name="htp", bufs=1) as htp, \
             tc.tile_pool(name="ps1", bufs=2, space="PSUM") as ps1p, \
             tc.tile_pool(name="ps2", bufs=2, space="PSUM") as ps2p, \
             tc.tile_pool(name="pst", bufs=2, space="PSUM") as pstp:
            for c in range(NCHUNK):
                xg = ffn.tile([128, NBLK, D], BF16, tag="xg")
                nc.gpsimd.dma_gather(
                    out_ap=xg,
                    in_ap=x_bf16[:, :],
                    idxs_ap=dIdx[:, c * (CB // 16):(c + 1) * (CB // 16)],
                    num_idxs=CB,
                    num_idxs_reg=CB,
                    elem_size=D,
                    transpose=False)
                xTb = ffn.tile([128, DCH, CB], BF16, tag="xTb")
                for dch in range(DCH):
                    psT = pstp.tile([128, CB], BF16, tag="psT")
                    for blk in range(NBLK):
                        nc.tensor.transpose(
                            psT[:, blk * 128:(blk + 1) * 128],
                            xg[:, blk, dch * 128:(dch + 1) * 128],
                            ident_b)
                    nc.vector.tensor_copy(xTb[:, dch, :], psT)
                hT = htp.tile([128, HCH, CB], BF16, tag="hT")
                for j in range(HCH):
                    ps1 = ps1p.tile([128, CB], FP32, tag="ps1")
                    for dch in range(DCH):
                        nc.tensor.matmul(
                            ps1, lhsT=W1s[:, dch, j * 128:(j + 1) * 128],
                            rhs=xTb[:, dch, :],
                            start=(dch == 0), stop=(dch == DCH - 1))
                    sgt = ffn.tile([128, CB], FP32, tag="sgt")
                    nc.scalar.activation(sgt, ps1, AF.Sigmoid,
                                         bias=b1s[:, j:j + 1])
                    nc.vector.scalar_tensor_tensor(
                        hT[:, j, :], ps1, b1s[:, j:j + 1], sgt,
                        op0=ALU.add, op1=ALU.mult)
                for cs in range(NBLK):
                    osb = ffn.tile([128, D], BF16, tag="osb")
                    for dh in range(2):
                        ps2 = ps2p.tile([128, 512], FP32, tag="ps2")
                        for j in range(HCH):
                            nc.tensor.matmul(
                                ps2,
                                lhsT=hT[:, j, cs * 128:(cs + 1) * 128],
                                rhs=W2s[:, j, dh * 512:(dh + 1) * 512],
                                start=(j == 0), stop=False)
                        nc.tensor.matmul(
                            ps2, lhsT=ones1b,
                            rhs=b2s[:, dh * 512:(dh + 1) * 512],
                            start=False, stop=True)
                        nc.vector.tensor_copy(osb[:, dh * 512:(dh + 1) * 512],
                                              ps2)
                    if c == NCHUNK - 1 and cs == NBLK - 1:
                        # ZSLOT row (expert-0 slot CAPP-1) must be zero
                        nc.vector.tensor_scalar(
                            osb, osb, zmask, None, op0=ALU.mult)
                    nc.sync.dma_start(
                        out_e[(c * NBLK + cs) * 128:
                              (c * NBLK + cs + 1) * 128, :],
                        osb)
                if AG_CHUNKED:
                    if c < NCHUNK - 1:
                        nc.gpsimd.collective_compute(
                            "AllGather", ALU.bypass,
                            replica_groups=[list(range(NC))],
                            ins=[out_e[c * CB:(c + 1) * CB, :]],
                            outs=[ag_c[c].opt()])
                        nc.sync.dma_start(all_out[c], ag_c[c][:, :, :])
                    else:
                        hb = CB // 2
                        for h in range(2):
                            r0 = c * CB + h * hb
                            nc.gpsimd.collective_compute(
                                "AllGather", ALU.bypass,
                                replica_groups=[list(range(NC))],
                                ins=[out_e[r0:r0 + hb, :]],
                                outs=[ag_h[h].opt()])
                            nc.sync.dma_start(
                                all_out[c][:, h * hb:(h + 1) * hb, :],
                                ag_h[h][:, :, :])
            if not AG_CHUNKED:
                nc.gpsimd.collective_compute(
                    "AllGather", ALU.bypass,
                    replica_groups=[list(range(NC))],
                    ins=[out_e.opt()], outs=[all_out.opt()])
            if dbg:
                nc.sync.dma_start(dbg_oe[:, :], out_e)

        # ---------- P4: combine own shard ----------
        with tc.tile_pool(name="comb", bufs=1) as comb:
            if AG_CHUNKED:
                allv = all_out.rearrange("n r c d -> (n r c) d")
            else:
                allv = all_out.rearrange("r c d -> (r c) d")
            GC = 8  # t-tiles per gather (1024 idxs)
            for t0 in range(0, TT, GC):
                gk = []
                for k in range(K):
                    gkt = comb.tile([128, GC, D], BF16, tag=f"gk{k}")
                    gk.append(gkt)
                    nc.gpsimd.dma_gather(
                        out_ap=gkt,
                        in_ap=allv,
                        idxs_ap=ci16[:, (k * TT + t0) * 8:
                                     (k * TT + t0 + GC) * 8],
                        num_idxs=GC * 128,
                        num_idxs_reg=GC * 128,
                        elem_size=D,
                        transpose=False)
                for tr in range(GC):
                    t = t0 + tr
                    ysb = comb.tile([128, D], FP32, tag="ysb")
                    y2 = comb.tile([128, D], FP32, tag="y2")
                    nc.vector.tensor_scalar(
                        ysb, gk[0][:, tr, :], g1o[:, t:t + 1], None,
                        op0=ALU.mult)
                    nc.vector.tensor_scalar(
                        y2, gk[1][:, tr, :], g2o[:, t:t + 1], None,
                        op0=ALU.mult)
                    nc.vector.tensor_tensor(ysb, ysb, y2, ALU.add)
                    nc.sync.dma_start(y_out[t * 128:(t + 1) * 128, :], ysb)

        top.close()

    nc.compile()
    return nc


# ---------------- host-side staging ----------------

def bfloat16_np():
    import ml_dtypes
    return ml_dtypes.bfloat16


def stage_inputs(cfg: Cfg, x, Wr, br, W1, b1, W2, b2):
    """x: [N, D] fp32; returns list of per-core input dicts."""
    E, D, H, TPC, NC = cfg.E, cfg.D, cfg.H, cfg.TPC, cfg.ncores
    DCH, HCH, TT = cfg.DCH, cfg.HCH, cfg.TT
    x = np.ascontiguousarray(x, np.float32)
    x_bf = x.astype(bfloat16_np())
    ltri = np.tril(np.ones((128, 128), np.float32), -1).astype(bfloat16_np())
    in_maps = []
    for r in range(NC):
        shard = x[r * TPC:(r + 1) * TPC]
        m = {
            "xT_shard": np.ascontiguousarray(shard.T),
            "x_bf16": x_bf,
            "Wr_in": np.ascontiguousarray(
                Wr.reshape(DCH, 128, E).transpose(1, 0, 2)).astype(np.float32),
            "br_in": br.reshape(1, E).astype(np.float32),
            "W1_in": np.ascontiguousarray(
                W1[r].reshape(DCH, 128, H).transpose(1, 0, 2)
            ).astype(bfloat16_np()),
            "W2_in": np.ascontiguousarray(
                W2[r].reshape(HCH, 128, D).transpose(1, 0, 2)
            ).astype(bfloat16_np()),
            "b1_in": np.ascontiguousarray(
                b1[r].reshape(HCH, 128).T).astype(np.float32),
            "b2_in": b2[r].reshape(1, D).astype(np.float32).astype(
                bfloat16_np()),
            "ltri_in": ltri,
            "rk_in": np.full((128, 1), r * TT, np.float32),
        }
        in_maps.append(m)
    return in_maps


# ---------------- problem binding ----------------

import math as _math

B, T = 8, 2048
_N = B * T
_D = 1024
_CAP = int(_math.ceil(1.2 * _N / 8))  # 2458

_CACHE = {}


def _get_nc():
    if "nc" not in _CACHE:
        cfg = Cfg(D=_D, H=4096, TPC=_N // 8, cap=_CAP, CAPP=2560, CB=512)
        _CACHE["cfg"] = cfg
        _CACHE["nc"] = build(cfg, dbg=bool(int(os.environ.get("KDBG", "0"))))
    return _CACHE["cfg"], _CACHE["nc"]


TRACE = False
_LAST_EXEC_NS = None
_LAST_RES = None


def kernel(x_btd, Wr, br, W1, b1, W2, b2):
    from concourse.bass_utils import run_bass_kernel_spmd

    global _LAST_EXEC_NS, _LAST_RES
    cfg, nc = _get_nc()
    x = np.ascontiguousarray(np.asarray(x_btd), np.float32).reshape(_N, _D)
    in_maps = stage_inputs(
        cfg, x, np.asarray(Wr), np.asarray(br), np.asarray(W1),
        np.asarray(b1), np.asarray(W2), np.asarray(b2))
    if TRACE:
        import shutil
        tdir = "/root/problem/work/trace"
        shutil.rmtree(tdir, ignore_errors=True)
        os.makedirs(tdir, exist_ok=True)
        res = run_bass_kernel_spmd(nc, in_maps, list(range(8)), trace=True,
                                   trace_cores=list(range(8)), tmpdir=tdir)
        _LAST_RES = res
        if getattr(res, "exec_time_ns", None):
            _LAST_EXEC_NS = res.exec_time_ns
    else:
        res = run_bass_kernel_spmd(nc, in_maps, list(range(8)))
        _LAST_RES = res
    ys = [res.results[r]["y_out"] for r in range(8)]
    y = np.concatenate(ys, axis=0).astype(np.float32)
    return y.reshape(B, T, _D)


# revision 55
# speedup vs baseline: 1.0761x; 1.0761x over previous
"""TRN2 Bass kernel for nn_MoEPositionwiseFFN: kernel(**inputs) -> np.ndarray.

v2: expert-parallel MoE FFN without dynamic-DMA dispatch scatter.

Per core r (= expert r):
  P1  router on own 2048 tokens (fp32), AllGather 4 routing planes.
  P2  replicated capacity selection (threshold bisection) -> keep/pos.
  P2.5 own-token extraction; slot->token dispatch table built with
       one-hot rank-1 matmuls into PSUM; ReduceScatter(add) routes each
       expert its [CAPP] slice. Combine indices built via PE transposes.
  P3  expert FFN on 2560 gathered rows (row dma_gather + PE transpose),
       output AllGather chunked (5x) to overlap with compute.
  P4  combine: gather 2 rows/token from all_out, gate-weighted sum
       (gates applied combine-side; ZSLOT row zeroed explicitly).
"""

import os
import sys

for _p in ("/opt/trn_rl_repo", "/opt/pypackages"):
    if _p not in sys.path:
        sys.path.insert(0, _p)


from dataclasses import dataclass

import numpy as np

import concourse.bass as bass
import concourse.bacc as bacc
import concourse.tile as tile
import concourse.mybir as mybir
from concourse.masks import make_identity

FP32 = mybir.dt.float32
BF16 = mybir.dt.bfloat16
I32 = mybir.dt.int32
I16 = mybir.dt.int16
U16 = mybir.dt.uint16
AF = mybir.ActivationFunctionType
ALU = mybir.AluOpType
AX = mybir.AxisListType


@dataclass
class Cfg:
    ncores: int = 8
    E: int = 8
    K: int = 2
    D: int = 1024
    H: int = 4096
    TPC: int = 2048          # tokens per core
    cap: int = 2458          # reference capacity
    CAPP: int = 2560         # padded capacity (= NCHUNK*CB)
    CB: int = 512            # FFN chunk / AllGather chunk (tokens)
    NBIS: int = 26           # bisection iterations (covers 0x3C000000..0x3F800000)

    @property
    def N(self):
        return self.ncores * self.TPC

    @property
    def TT(self):
        return self.TPC // 128  # token tiles per core (16)

    @property
    def M(self):
        return self.N // 128    # global token groups (128)

    @property
    def DCH(self):
        return self.D // 128

    @property
    def HCH(self):
        return self.H // 128

    @property
    def NCHUNK(self):
        return self.CAPP // self.CB

    @property
    def FW(self):
        return self.CAPP // 16  # dispatch-table free width (160)


def build(cfg: Cfg, dbg: bool = False):
    E, K, D, H = cfg.E, cfg.K, cfg.D, cfg.H
    TPC, TT, M, N = cfg.TPC, cfg.TT, cfg.M, cfg.N
    DCH, HCH = cfg.DCH, cfg.HCH
    CAP, CAPP, CB, NCHUNK, FW = cfg.cap, cfg.CAPP, cfg.CB, cfg.NCHUNK, cfg.FW
    NC = cfg.ncores
    assert E == NC == 8 and K == 2
    assert CAPP == NCHUNK * CB and CAPP % 16 == 0 and CB % 128 == 0
    NBLK = CB // 128          # 128-token blocks per chunk (4)
    # combine flat row for dropped assignments: chunk layout row of
    # (expert 0, slot CAPP-1), guaranteed zeroed.
    ZC = (CAPP - 1) // CB
    ZOFF = ZC * (NC * CB) + 0 * CB + ((CAPP - 1) - ZC * CB)

    nc = bacc.Bacc("TRN2", target_bir_lowering=False, debug=False,
                   num_devices=NC)

    # ---- external inputs (per-core staged by host) ----
    xT_shard = nc.dram_tensor("xT_shard", [D, TPC], FP32, kind="ExternalInput")
    x_bf16 = nc.dram_tensor("x_bf16", [N, D], BF16, kind="ExternalInput")
    Wr_in = nc.dram_tensor("Wr_in", [128, DCH, E], FP32, kind="ExternalInput")
    br_in = nc.dram_tensor("br_in", [1, E], FP32, kind="ExternalInput")
    W1_in = nc.dram_tensor("W1_in", [128, DCH, H], BF16, kind="ExternalInput")
    W2_in = nc.dram_tensor("W2_in", [128, HCH, D], BF16, kind="ExternalInput")
    b1_in = nc.dram_tensor("b1_in", [128, HCH], FP32, kind="ExternalInput")
    b2_in = nc.dram_tensor("b2_in", [1, D], BF16, kind="ExternalInput")
    ltri_in = nc.dram_tensor("ltri_in", [128, 128], BF16, kind="ExternalInput")
    rk_in = nc.dram_tensor("rk_in", [128, 1], FP32, kind="ExternalInput")

    # ---- external output ----
    y_out = nc.dram_tensor("y_out", [TPC, D], FP32, kind="ExternalOutput")

    if dbg:
        dbg_rta = nc.dram_tensor("dbg_rta", [4, 128, M], FP32,
                                 kind="ExternalOutput")
        dbg_pos = nc.dram_tensor("dbg_pos", [128, E * M], FP32,
                                 kind="ExternalOutput")
        dbg_keep = nc.dram_tensor("dbg_keep", [128, E * M], FP32,
                                  kind="ExternalOutput")
        dbg_tab = nc.dram_tensor("dbg_tab", [16, 2 * FW], FP32,
                                 kind="ExternalOutput")
        dbg_cidx = nc.dram_tensor("dbg_cidx", [128, K * TT * 8], I16,
                                  kind="ExternalOutput")
        dbg_didx = nc.dram_tensor("dbg_didx", [128, FW], I16,
                                  kind="ExternalOutput")
        dbg_oe = nc.dram_tensor("dbg_oe", [CAPP, D], BF16,
                                kind="ExternalOutput")

    with tile.TileContext(nc) as tc:
        rank_sp = nc.partition_id()

        import contextlib
        top = contextlib.ExitStack()
        cpool = top.enter_context(tc.tile_pool(name="const", bufs=1))
        wts = top.enter_context(tc.tile_pool(name="wts", bufs=1))
        keepp = top.enter_context(tc.tile_pool(name="keepp", bufs=1))
        dramp = top.enter_context(tc.tile_pool(name="dramp", bufs=1,
                                               space="DRAM"))

        # ---- DRAM scratch ----
        rt_local = dramp.tile([4, 128, TT], FP32, tag="rt_local")
        rt_all = dramp.tile([NC, 4, 128, TT], FP32, tag="rt_all",
                            addr_space="Shared")
        tab_dram = dramp.tile([128, 2 * FW], FP32, tag="tab_dram")
        tab_all = dramp.tile([NC, 128, 2 * FW], FP32, tag="tab_all",
                             addr_space="Shared")
        out_e = dramp.tile([CAPP, D], BF16, tag="out_e")
        AG_CHUNKED = True
        if AG_CHUNKED:
            all_out = dramp.tile([NCHUNK, NC, CB, D], BF16, tag="all_out")
        else:
            all_out = dramp.tile([NC, CAPP, D], BF16, tag="all_out",
                                 addr_space="Shared")

        # ---- weights: DMA starts immediately, overlaps P1/P2 ----
        W1s = wts.tile([128, DCH, H], BF16, tag="W1s")
        nc.sync.dma_start(W1s, W1_in[:, :, :])
        W2s = wts.tile([128, HCH, D], BF16, tag="W2s")
        nc.sync.dma_start(W2s, W2_in[:, :, :])
        b1s = wts.tile([128, HCH], FP32, tag="b1s")
        nc.sync.dma_start(b1s, b1_in[:, :])
        b2s = wts.tile([1, D], BF16, tag="b2s")
        nc.sync.dma_start(b2s, b2_in[:, :])

        # ---- constants ----
        ident_b = cpool.tile([128, 128], BF16, tag="ident_b")
        make_identity(nc, ident_b)
        ident_f = cpool.tile([128, 128], FP32, tag="ident_f")
        make_identity(nc, ident_f)
        ltri = cpool.tile([128, 128], BF16, tag="ltri")
        nc.sync.dma_start(ltri, ltri_in[:, :])
        wr_sb = cpool.tile([128, DCH, E], FP32, tag="wr")
        nc.sync.dma_start(wr_sb, Wr_in[:, :, :])
        br_sb = cpool.tile([1, E], FP32, tag="br")
        nc.sync.dma_start(br_sb, br_in[:, :])
        rk_sb = cpool.tile([128, 1], FP32, tag="rk_sb")
        nc.sync.dma_start(rk_sb, rk_in[:, :])
        ones1f = cpool.tile([1, 128], FP32, tag="ones1f")
        nc.vector.memset(ones1f, 1.0)
        ones1b = cpool.tile([1, 128], BF16, tag="ones1b")
        nc.vector.memset(ones1b, 1.0)
        ones128b = cpool.tile([128, 128], BF16, tag="ones128b")
        nc.vector.memset(ones128b, 1.0)
        one_i = cpool.tile([128, E], I32, tag="one_i")
        nc.vector.memset(one_i, 1)
        # iota along free: F160[p, j] = j ; F128 = F160[:, :128]
        it_i = cpool.tile([128, FW], I32, tag="it_i")
        nc.gpsimd.iota(it_i, pattern=[[1, FW]], base=0, channel_multiplier=0)
        F160 = cpool.tile([128, FW], FP32, tag="F160")
        nc.vector.tensor_copy(F160, it_i)
        F128 = F160[:, 0:128]
        # lovals[p, 0] = p
        lov_i = cpool.tile([128, 1], I32, tag="lov_i")
        nc.gpsimd.iota(lov_i, pattern=[[1, 1]], base=0, channel_multiplier=1)
        lovals = cpool.tile([128, 1], FP32, tag="lovals")
        nc.vector.tensor_copy(lovals, lov_i)
        # hival[p, t] = rank*TT + t   (token-group id of own tile t)
        tt_i = cpool.tile([128, TT], I32, tag="tt_i")
        nc.gpsimd.iota(tt_i, pattern=[[1, TT]], base=0, channel_multiplier=0)
        hival = cpool.tile([128, TT], FP32, tag="hival")
        nc.vector.tensor_copy(hival, tt_i)
        nc.vector.tensor_scalar(hival, hival, rk_sb, None, op0=ALU.add)
        # zmask[p] = 0 for p == 127 else 1 (ZSLOT row kill)
        zmask = cpool.tile([128, 1], FP32, tag="zmask")
        nc.vector.tensor_scalar(zmask, lovals, 127.0, None,
                                op0=ALU.not_equal)
        # tokv[p, t] = global token id of own (t, p) = (rk+t)*128 + p
        tokv = cpool.tile([128, TT], FP32, tag="tokv")
        nc.vector.tensor_scalar(tokv, hival, 128.0, lovals,
                                op0=ALU.mult, op1=ALU.add)

        # ---- persistent small tiles (survive into P3/P4) ----
        dIdx = keepp.tile([128, FW], I16, tag="dIdx")
        ci16 = keepp.tile([128, K * TT * 8], I16, tag="ci16")
        g1o = keepp.tile([128, TT], FP32, tag="g1o")
        g2o = keepp.tile([128, TT], FP32, tag="g2o")
        gdp = keepp.tile([128, NCHUNK * NBLK], FP32, tag="gdp")

        selstack = contextlib.ExitStack()
        sel = selstack.enter_context(tc.tile_pool(name="sel", bufs=1))
        lrp = selstack.enter_context(tc.tile_pool(name="lrp", bufs=2))
        psr = selstack.enter_context(
            tc.tile_pool(name="psr", bufs=2, space="PSUM"))
        pscnt = selstack.enter_context(
            tc.tile_pool(name="pscnt", bufs=2, space="PSUM"))
        pstab = selstack.enter_context(
            tc.tile_pool(name="pstab", bufs=1, space="PSUM"))
        pstr = selstack.enter_context(
            tc.tile_pool(name="pstr", bufs=1, space="PSUM"))
        xstack = contextlib.ExitStack()
        xpool = xstack.enter_context(tc.tile_pool(name="xpool", bufs=2))

        # ---------- P1: router on own shard ----------
        E_sb = sel.tile([128, TT, E], FP32, tag="E_sb")
        QT = TT // 4  # t-tiles per quarter
        for q4 in range(4):
            xq = xpool.tile([128, DCH, QT * 128], FP32, tag="xq")
            nc.sync.dma_start(
                xq,
                xT_shard[:, q4 * QT * 128:(q4 + 1) * QT * 128]
                .rearrange("(dch p) t -> p dch t", p=128))
            for tr in range(QT):
                t = q4 * QT + tr
                ps = psr.tile([128, E], FP32, tag="psr")
                for dch in range(DCH):
                    nc.tensor.matmul(
                        ps, lhsT=xq[:, dch, tr * 128:(tr + 1) * 128],
                        rhs=wr_sb[:, dch, :],
                        start=(dch == 0), stop=False)
                nc.tensor.matmul(ps, lhsT=ones1f, rhs=br_sb[:, :],
                                 start=False, stop=True)
                nc.scalar.activation(E_sb[:, t, :], ps, AF.Exp)
        xstack.close()
        selB = selstack.enter_context(tc.tile_pool(name="selB", bufs=1))
        # batched top-2 over the expert axis
        e8i = sel.tile([128, TT * 8], I16, tag="e8i")
        nc.gpsimd.iota(e8i, pattern=[[0, TT], [1, 8]], base=0,
                       channel_multiplier=0)
        e8f = sel.tile([128, TT, 8], FP32, tag="e8f")
        nc.vector.tensor_copy(e8f.rearrange("p a b -> p (a b)"), e8i)
        Z_sb = sel.tile([128, TT], FP32, tag="Z_sb")
        nc.vector.tensor_reduce(Z_sb, E_sb, AX.X, ALU.add)
        m1 = sel.tile([128, TT], FP32, tag="m1")
        nc.vector.tensor_reduce(m1, E_sb, AX.X, ALU.max)
        eqx = sel.tile([128, TT, 8], FP32, tag="eqx")
        nc.vector.tensor_tensor(
            eqx, E_sb, m1.unsqueeze(2).broadcast_to((128, TT, 8)),
            ALU.is_equal)
        tmp8 = sel.tile([128, TT, 8], FP32, tag="tmp8")
        nc.vector.tensor_tensor(tmp8, eqx, e8f, ALU.mult)
        P_i1 = sel.tile([128, TT], FP32, tag="P_i1")
        nc.vector.tensor_reduce(P_i1, tmp8, AX.X, ALU.add)
        nc.vector.tensor_scalar(tmp8, eqx, -1e30, None, op0=ALU.mult)
        nc.vector.tensor_tensor(E_sb, E_sb, tmp8, ALU.add)  # mask out top-1
        m2 = sel.tile([128, TT], FP32, tag="m2")
        nc.vector.tensor_reduce(m2, E_sb, AX.X, ALU.max)
        nc.vector.tensor_tensor(
            eqx, E_sb, m2.unsqueeze(2).broadcast_to((128, TT, 8)),
            ALU.is_equal)
        nc.vector.tensor_tensor(tmp8, eqx, e8f, ALU.mult)
        P_i2 = sel.tile([128, TT], FP32, tag="P_i2")
        nc.vector.tensor_reduce(P_i2, tmp8, AX.X, ALU.add)
        rZ = sel.tile([128, TT], FP32, tag="rZ")
        nc.vector.reciprocal(rZ, Z_sb)
        P_g1 = sel.tile([128, TT], FP32, tag="P_g1")
        P_g2 = sel.tile([128, TT], FP32, tag="P_g2")
        nc.vector.tensor_tensor(P_g1, m1, rZ, ALU.mult)
        nc.vector.tensor_tensor(P_g2, m2, rZ, ALU.mult)
        nc.sync.dma_start(rt_local[0], P_i1)
        nc.sync.dma_start(rt_local[1], P_g1)
        nc.sync.dma_start(rt_local[2], P_i2)
        nc.sync.dma_start(rt_local[3], P_g2)
        nc.gpsimd.collective_compute(
            "AllGather", ALU.bypass,
            replica_groups=[list(range(NC))],
            ins=[rt_local.opt()], outs=[rt_all.opt()])

        # ---------- P2: replicated selection ----------
        i1f = sel.tile([128, M], FP32, tag="i1f")
        g1f = sel.tile([128, M], FP32, tag="g1f")
        i2f = sel.tile([128, M], FP32, tag="i2f")
        g2f = sel.tile([128, M], FP32, tag="g2f")
        for q, dst in ((0, i1f), (1, g1f), (2, i2f), (3, g2f)):
            nc.sync.dma_start(
                dst.rearrange("p (r t) -> p r t", r=NC),
                rt_all[:, q, :, :].rearrange("r p t -> p r t"))
        if dbg:
            nc.sync.dma_start(dbg_rta[0], i1f)
            nc.sync.dma_start(dbg_rta[1], g1f)
            nc.sync.dma_start(dbg_rta[2], i2f)
            nc.sync.dma_start(dbg_rta[3], g2f)

        A_sb = selB.tile([128, E, M], FP32, tag="A_sb")
        tmpM = sel.tile([128, M], FP32, tag="tmpM")
        for e in range(E):
            nc.vector.scalar_tensor_tensor(
                A_sb[:, e, :], i1f, float(e), g1f,
                op0=ALU.is_equal, op1=ALU.mult)
            nc.vector.scalar_tensor_tensor(
                tmpM, i2f, float(e), g2f, op0=ALU.is_equal, op1=ALU.mult)
            nc.vector.tensor_tensor(A_sb[:, e, :], A_sb[:, e, :], tmpM,
                                    ALU.add)

        big = selB.tile([128, E, M], FP32, tag="big")
        cntp = sel.tile([128, E], FP32, tag="cntp")
        cntb = sel.tile([128, E], BF16, tag="cntb")
        cntf = sel.tile([128, E], FP32, tag="cntf")
        Ktgt = sel.tile([128, E], FP32, tag="Ktgt")
        lo = sel.tile([128, E], I32, tag="lo")
        hi = sel.tile([128, E], I32, tag="hi")
        mid = sel.tile([128, E], I32, tag="mid")
        condi = sel.tile([128, E], I32, tag="condi")

        nc.vector.tensor_scalar(big, A_sb, 0.0, None, op0=ALU.is_gt)
        nc.vector.tensor_reduce(cntp, big, AX.X, ALU.add)
        nc.vector.tensor_copy(cntb, cntp)
        pc = pscnt.tile([128, E], FP32, tag="pscnt")
        nc.tensor.matmul(pc, lhsT=ones128b, rhs=cntb, start=True, stop=True)
        nc.vector.tensor_scalar(Ktgt, pc, float(CAP), None, op0=ALU.min)

        zerosM = selB.tile([128, M], FP32, tag="zerosM")
        nc.vector.memset(zerosM, 0.0)
        nc.vector.memset(lo, 0x3C000000)
        nc.vector.memset(hi, 0x3F800000)
        for it in range(cfg.NBIS):
            nc.vector.tensor_tensor(mid, lo, hi, ALU.add)
            nc.vector.tensor_tensor(mid, mid, one_i,
                                    ALU.logical_shift_right)
            midf = mid.bitcast(FP32)
            for e in range(E):
                nc.vector.scalar_tensor_tensor(
                    big[:, e, :], A_sb[:, e, :], midf[:, e:e + 1], zerosM,
                    op0=ALU.is_gt, op1=ALU.add,
                    accum_out=cntp[:, e:e + 1])
            nc.vector.tensor_copy(cntb, cntp)
            pc = pscnt.tile([128, E], FP32, tag="pscnt")
            nc.tensor.matmul(pc, lhsT=ones128b, rhs=cntb, start=True,
                             stop=True)
            nc.vector.tensor_copy(cntf, pc)
            nc.vector.tensor_tensor(condi, cntf, Ktgt, ALU.is_ge)
            nc.vector.copy_predicated(lo, condi, mid)
            nc.vector.tensor_tensor(condi, cntf, Ktgt, ALU.is_lt)
            nc.vector.copy_predicated(hi, condi, mid)

        thrf = lo.bitcast(FP32)
        keepf = selB.tile([128, E, M], FP32, tag="keepf")
        nc.vector.tensor_tensor(
            keepf, A_sb, thrf.unsqueeze(2).broadcast_to((128, E, M)),
            ALU.is_gt)

        rp = selB.tile([128, E, M], FP32, tag="rp")
        for e in range(E):
            nc.vector.tensor_tensor_scan(
                rp[:, e, :], keepf[:, e, :], zerosM, initial=0.0,
                op0=ALU.add, op1=ALU.add)
        totb = sel.tile([128, E], BF16, tag="totb")
        nc.vector.tensor_copy(totb, rp[:, :, M - 1])
        pe_x = pscnt.tile([128, E], FP32, tag="pscnt")
        nc.tensor.matmul(pe_x, lhsT=ltri, rhs=totb, start=True, stop=True)
        excl = sel.tile([128, E], FP32, tag="excl")
        nc.vector.tensor_copy(excl, pe_x)
        pos = selB.tile([128, E, M], FP32, tag="pos")
        nc.vector.tensor_tensor(pos, rp, keepf, ALU.subtract)
        nc.vector.tensor_tensor(
            pos, pos, excl.unsqueeze(2).broadcast_to((128, E, M)),
            ALU.add)
        if dbg:
            nc.sync.dma_start(dbg_pos[:, :],
                              pos.rearrange("p e m -> p (e m)"))
            nc.sync.dma_start(dbg_keep[:, :],
                              keepf.rearrange("p e m -> p (e m)"))

        # ---------- P2.5: own-token extraction + dispatch table ----------
        own0 = bass.ds(rank_sp * TT, TT)
        c15 = sel.tile([128, TT], I32, tag="c15")
        nc.vector.memset(c15, 15)
        c4 = sel.tile([128, TT], I32, tag="c4")
        nc.vector.memset(c4, 4)
        c511 = sel.tile([128, TT], I32, tag="c511")
        nc.vector.memset(c511, 511)
        c9 = sel.tile([128, TT], I32, tag="c9")
        nc.vector.memset(c9, 9)
        pl_k = []      # [128, TT] fp32 per k: table row  e*16 + pos%16
        fs_k = []      # [128, TT] fp32 per k: table col  pos//16 (999=dead)
        offall = sel.tile([128, K * TT], FP32, tag="offall")
        tmpT = sel.tile([128, TT], FP32, tag="tmpT")
        for k in range(K):
            ikf = i1f if k == 0 else i2f
            gkf = g1f if k == 0 else g2f
            go = g1o if k == 0 else g2o
            nc.vector.tensor_copy(go, gkf[:, own0])
            iko = sel.tile([128, TT], FP32, tag=f"iko{k}")
            nc.vector.tensor_copy(iko, ikf[:, own0])
            posk = sel.tile([128, TT], FP32, tag=f"posk{k}")
            keepk = sel.tile([128, TT], FP32, tag=f"keepk{k}")
            first = True
            for e in range(E):
                dst = posk if first else tmpT
                nc.vector.scalar_tensor_tensor(
                    dst, iko, float(e), pos[:, e, own0],
                    op0=ALU.is_equal, op1=ALU.mult)
                if not first:
                    nc.vector.tensor_tensor(posk, posk, tmpT, ALU.add)
                first = False
            first = True
            for e in range(E):
                dst = keepk if first else tmpT
                nc.vector.scalar_tensor_tensor(
                    dst, iko, float(e), keepf[:, e, own0],
                    op0=ALU.is_equal, op1=ALU.mult)
                if not first:
                    nc.vector.tensor_tensor(keepk, keepk, tmpT, ALU.add)
                first = False
            keepi = sel.tile([128, TT], I32, tag=f"keepi{k}")
            nc.vector.tensor_copy(keepi, keepk)
            # table coords (integer split of pos: %16 and //16)
            posI = sel.tile([128, TT], I32, tag=f"posI{k}")
            nc.vector.tensor_copy(posI, posk)
            tmpI = sel.tile([128, TT], I32, tag=f"tmpI{k}")
            nc.vector.tensor_tensor(tmpI, posI, c15, ALU.bitwise_and)
            qo = sel.tile([128, TT], FP32, tag=f"qo{k}")
            nc.vector.tensor_copy(qo, tmpI)
            nc.vector.tensor_tensor(tmpI, posI, c4, ALU.logical_shift_right)
            fo = sel.tile([128, TT], FP32, tag=f"fo{k}")
            nc.vector.tensor_copy(fo, tmpI)
            plo = sel.tile([128, TT], FP32, tag=f"plo{k}")
            nc.vector.scalar_tensor_tensor(
                plo, iko, 16.0, qo, op0=ALU.mult, op1=ALU.add)
            fsel = sel.tile([128, TT], FP32, tag=f"fsel{k}")
            nc.vector.memset(fsel, 999.0)
            nc.vector.copy_predicated(fsel, keepi, fo)
            pl_k.append(plo)
            fs_k.append(fsel)
            off = sel.tile([128, TT], FP32, tag=f"off{k}")
            if AG_CHUNKED:
                # combine flat row (chunk layout): c = pos//CB;
                # off = c*(NC*CB) + ik*CB + pos%CB ; dropped -> ZOFF
                nc.vector.tensor_tensor(tmpI, posI, c511, ALU.bitwise_and)
                m5 = sel.tile([128, TT], FP32, tag=f"m5{k}")
                nc.vector.tensor_copy(m5, tmpI)
                nc.vector.tensor_tensor(tmpI, posI, c9,
                                        ALU.logical_shift_right)
                cdv = sel.tile([128, TT], FP32, tag=f"cdv{k}")
                nc.vector.tensor_copy(cdv, tmpI)
                nc.vector.tensor_scalar(cdv, cdv, float(NC * CB), None,
                                        op0=ALU.mult)
                nc.vector.scalar_tensor_tensor(
                    off, iko, float(CB), m5, op0=ALU.mult, op1=ALU.add)
                nc.vector.tensor_tensor(off, off, cdv, ALU.add)
                zoff = float(ZOFF)
            else:
                # off = ik*CAPP + pos ; dropped -> expert0 slot CAPP-1
                nc.vector.scalar_tensor_tensor(
                    off, iko, float(CAPP), posk, op0=ALU.mult, op1=ALU.add)
                zoff = float(CAPP - 1)
            offd = sel.tile([128, TT], FP32, tag=f"offd{k}")
            nc.vector.memset(offd, zoff)
            nc.vector.copy_predicated(offd, keepi, off)
            nc.vector.tensor_copy(offall[:, k * TT:(k + 1) * TT], offd)

        # table build: psTab[row, :] += sum over items of onehot outer
        # (fp32 planes: [0:FW] token id, [FW:2FW] gate)
        psTab = pstab.tile([128, 2 * FW], FP32, tag="psTab")
        nck = 0
        for k in range(K):
            go = g1o if k == 0 else g2o
            for t in range(TT):
                L = lrp.tile([128, 128], FP32, tag="L")
                nc.vector.tensor_scalar(
                    L, F128, pl_k[k][:, t:t + 1], None, op0=ALU.is_equal)
                R = lrp.tile([128, 2 * FW], FP32, tag="R")
                nc.vector.tensor_scalar(
                    R[:, 0:FW], F160, fs_k[k][:, t:t + 1], tokv[:, t:t + 1],
                    op0=ALU.is_equal, op1=ALU.mult)
                nc.vector.tensor_scalar(
                    R[:, FW:2 * FW], F160, fs_k[k][:, t:t + 1],
                    go[:, t:t + 1], op0=ALU.is_equal, op1=ALU.mult)
                nc.tensor.matmul(psTab, lhsT=L, rhs=R,
                                 start=(nck == 0), stop=(nck == K * TT - 1))
                nck += 1
        tabsb = selB.tile([128, 2 * FW], FP32, tag="tabsb")
        nc.vector.tensor_copy(tabsb, psTab)
        nc.sync.dma_start(tab_dram, tabsb)
        nc.gpsimd.collective_compute(
            "AllGather", ALU.bypass,
            replica_groups=[list(range(NC))],
            ins=[tab_dram.opt()], outs=[tab_all.opt()])

        # readback own expert's 16 rows from each core's table and sum
        own16 = bass.ds(rank_sp * 16, 16)
        tabs = selB.tile([16, 2 * FW], FP32, tag="tabs")
        tabr = selB.tile([16, NC, 2 * FW], FP32, tag="tabr")
        for r in range(NC):
            nc.sync.dma_start(tabr[:, r, :], tab_all[r, own16, :])
        nc.vector.tensor_tensor(tabs, tabr[:, 0, :], tabr[:, 1, :], ALU.add)
        for r in range(2, NC):
            nc.vector.tensor_tensor(tabs, tabs, tabr[:, r, :], ALU.add)
        if dbg:
            nc.sync.dma_start(dbg_tab, tabs)
        dI16 = sel.tile([16, FW], I16, tag="dI16")
        nc.vector.tensor_copy(dI16, tabs[:, 0:FW])
        # per-slot gates, relayout [16q, 160f] -> [128 = (f%8)*16+q, f//8]
        gview = tabs[:, FW:2 * FW].rearrange("q (fd fm) -> q fd fm", fm=8)
        for fm in range(8):
            nc.sync.dma_start(gdp[fm * 16:(fm + 1) * 16, :],
                              gview[:, :, fm])
        for g in range(8):
            nc.sync.dma_start(dIdx[g * 16:(g + 1) * 16, :], dI16)

        # combine idx relayout via PE transposes:
        # cidxf[pl, (k t), ph] = offall[ph*16+pl, (k t)]
        psO = pstr.tile([32, 128], FP32, tag="psO")
        nc.tensor.transpose(psO, offall, ident_f)
        T1 = sel.tile([32, 128], FP32, tag="T1")
        nc.vector.tensor_copy(T1, psO)
        cidxf = sel.tile([128, K * TT, 8], FP32, tag="cidxf")
        for ph in range(8):
            psP = pstr.tile([16, 32], FP32, tag="psP")
            nc.tensor.transpose(psP, T1[:, ph * 16:(ph + 1) * 16],
                                ident_f[0:32, 0:32])
            nc.vector.tensor_copy(cidxf[0:16, :, ph], psP)
        cs16 = sel.tile([16, K * TT * 8], I16, tag="cs16")
        nc.vector.tensor_copy(
            cs16, cidxf[0:16].rearrange("p a b -> p (a b)"))
        for g in range(8):
            nc.sync.dma_start(ci16[g * 16:(g + 1) * 16, :], cs16)
        if dbg:
            nc.sync.dma_start(dbg_cidx, ci16)
            nc.sync.dma_start(dbg_didx, dIdx)

        selstack.close()

        # ---------- P3: expert FFN ----------
        with tc.tile_pool(name="ffn", bufs=2) as ffn, \
             tc.tile_pool(name="htp", bufs=1) as htp, \
             tc.tile_pool(name="ps1", bufs=2, space="PSUM") as ps1p, \
             tc.tile_pool(name="ps2", bufs=2, space="PSUM") as ps2p, \
             tc.tile_pool(name="pst", bufs=2, space="PSUM") as pstp:
            for c in range(NCHUNK):
                xg = ffn.tile([128, NBLK, D], BF16, tag="xg")
                nc.gpsimd.dma_gather(
                    out_ap=xg,
                    in_ap=x_bf16[:, :],
                    idxs_ap=dIdx[:, c * (CB // 16):(c + 1) * (CB // 16)],
                    num_idxs=CB,
                    num_idxs_reg=CB,
                    elem_size=D,
                    transpose=False)
                xTb = ffn.tile([128, DCH, CB], BF16, tag="xTb")
                for dch in range(DCH):
                    psT = pstp.tile([128, CB], BF16, tag="psT")
                    for blk in range(NBLK):
                        nc.tensor.transpose(
                            psT[:, blk * 128:(blk + 1) * 128],
                            xg[:, blk, dch * 128:(dch + 1) * 128],
                            ident_b)
                    nc.vector.tensor_copy(xTb[:, dch, :], psT)
                hT = htp.tile([128, HCH, CB], BF16, tag="hT")
                for j in range(HCH):
                    ps1 = ps1p.tile([128, CB], FP32, tag="ps1")
                    for dch in range(DCH):
                        nc.tensor.matmul(
                            ps1, lhsT=W1s[:, dch, j * 128:(j + 1) * 128],
                            rhs=xTb[:, dch, :],
                            start=(dch == 0), stop=(dch == DCH - 1))
                    sgt = ffn.tile([128, CB], FP32, tag="sgt")
                    nc.scalar.activation(sgt, ps1, AF.Sigmoid,
                                         bias=b1s[:, j:j + 1])
                    nc.vector.scalar_tensor_tensor(
                        hT[:, j, :], ps1, b1s[:, j:j + 1], sgt,
                        op0=ALU.add, op1=ALU.mult)
                for cs in range(NBLK):
                    osb = ffn.tile([128, D], BF16, tag="osb")
                    for dh in range(2):
                        ps2 = ps2p.tile([128, 512], FP32, tag="ps2")
                        for j in range(HCH):
                            nc.tensor.matmul(
                                ps2,
                                lhsT=hT[:, j, cs * 128:(cs + 1) * 128],
                                rhs=W2s[:, j, dh * 512:(dh + 1) * 512],
                                start=(j == 0), stop=False)
                        nc.tensor.matmul(
                            ps2, lhsT=ones1b,
                            rhs=b2s[:, dh * 512:(dh + 1) * 512],
                            start=False, stop=True)
                        blk = c * NBLK + cs
                        nc.vector.tensor_scalar(
                            osb[:, dh * 512:(dh + 1) * 512], ps2,
                            gdp[:, blk:blk + 1], None, op0=ALU.mult)
                    nc.sync.dma_start(
                        out_e[(c * NBLK + cs) * 128:
                              (c * NBLK + cs + 1) * 128, :],
                        osb)
                if AG_CHUNKED:
                    nc.gpsimd.collective_compute(
                        "AllGather", ALU.bypass,
                        replica_groups=[list(range(NC))],
                        ins=[out_e[c * CB:(c + 1) * CB, :]],
                        outs=[all_out[c]])
            if not AG_CHUNKED:
                nc.gpsimd.collective_compute(
                    "AllGather", ALU.bypass,
                    replica_groups=[list(range(NC))],
                    ins=[out_e.opt()], outs=[all_out.opt()])
            if dbg:
                nc.sync.dma_start(dbg_oe[:, :], out_e)

        # ---------- P4: combine own shard ----------
        with tc.tile_pool(name="comb", bufs=1) as comb:
            if AG_CHUNKED:
                allv = all_out.rearrange("n r c d -> (n r c) d")
            else:
                allv = all_out.rearrange("r c d -> (r c) d")
            GC = 8  # t-tiles per gather (1024 idxs)
            for t0 in range(0, TT, GC):
                gk = []
                for k in range(K):
                    gkt = comb.tile([128, GC, D], BF16, tag=f"gk{k}")
                    gk.append(gkt)
                    nc.gpsimd.dma_gather(
                        out_ap=gkt,
                        in_ap=allv,
                        idxs_ap=ci16[:, (k * TT + t0) * 8:
                                     (k * TT + t0 + GC) * 8],
                        num_idxs=GC * 128,
                        num_idxs_reg=GC * 128,
                        elem_size=D,
                        transpose=False)
                for tr in range(GC):
                    t = t0 + tr
                    ysb = comb.tile([128, D], FP32, tag="ysb")
                    nc.vector.tensor_tensor(ysb, gk[0][:, tr, :],
                                            gk[1][:, tr, :], ALU.add)
                    nc.sync.dma_start(y_out[t * 128:(t + 1) * 128, :], ysb)

        top.close()

    nc.compile()
    return nc


# ---------------- host-side staging ----------------

def bfloat16_np():
    import ml_dtypes
    return ml_dtypes.bfloat16


def stage_inputs(cfg: Cfg, x, Wr, br, W1, b1, W2, b2):
    """x: [N, D] fp32; returns list of per-core input dicts."""
    E, D, H, TPC, NC = cfg.E, cfg.D, cfg.H, cfg.TPC, cfg.ncores
    DCH, HCH, TT = cfg.DCH, cfg.HCH, cfg.TT
    x = np.ascontiguousarray(x, np.float32)
    x_bf = x.astype(bfloat16_np())
    ltri = np.tril(np.ones((128, 128), np.float32), -1).astype(bfloat16_np())
    in_maps = []
    for r in range(NC):
        shard = x[r * TPC:(r + 1) * TPC]
        m = {
            "xT_shard": np.ascontiguousarray(shard.T),
            "x_bf16": x_bf,
            "Wr_in": np.ascontiguousarray(
                Wr.reshape(DCH, 128, E).transpose(1, 0, 2)).astype(np.float32),
            "br_in": br.reshape(1, E).astype(np.float32),
            "W1_in": np.ascontiguousarray(
                W1[r].reshape(DCH, 128, H).transpose(1, 0, 2)
            ).astype(bfloat16_np()),
            "W2_in": np.ascontiguousarray(
                W2[r].reshape(HCH, 128, D).transpose(1, 0, 2)
            ).astype(bfloat16_np()),
            "b1_in": np.ascontiguousarray(
                b1[r].reshape(HCH, 128).T).astype(np.float32),
            "b2_in": b2[r].reshape(1, D).astype(np.float32).astype(
                bfloat16_np()),
            "ltri_in": ltri,
            "rk_in": np.full((128, 1), r * TT, np.float32),
        }
        in_maps.append(m)
    return in_maps


# ---------------- problem binding ----------------

import math as _math

B, T = 8, 2048
_N = B * T
_D = 1024
_CAP = int(_math.ceil(1.2 * _N / 8))  # 2458

_CACHE = {}


def _get_nc():
    if "nc" not in _CACHE:
        cfg = Cfg(D=_D, H=4096, TPC=_N // 8, cap=_CAP, CAPP=2560, CB=512)
        _CACHE["cfg"] = cfg
        _CACHE["nc"] = build(cfg, dbg=bool(int(os.environ.get("KDBG", "0"))))
    return _CACHE["cfg"], _CACHE["nc"]


TRACE = False
_LAST_EXEC_NS = None
_LAST_RES = None


def kernel(x_btd, Wr, br, W1, b1, W2, b2):
    from concourse.bass_utils import run_bass_kernel_spmd

    global _LAST_EXEC_NS, _LAST_RES
    cfg, nc = _get_nc()
    x = np.ascontiguousarray(np.asarray(x_btd), np.float32).reshape(_N, _D)
    in_maps = stage_inputs(
        cfg, x, np.asarray(Wr), np.asarray(br), np.asarray(W1),
        np.asarray(b1), np.asarray(W2), np.asarray(b2))
    if TRACE:
        import shutil
        tdir = "/root/problem/work/trace"
        shutil.rmtree(tdir, ignore_errors=True)
        os.makedirs(tdir, exist_ok=True)
        res = run_bass_kernel_spmd(nc, in_maps, list(range(8)), trace=True,
                                   trace_cores=list(range(8)), tmpdir=tdir)
        _LAST_RES = res
        if getattr(res, "exec_time_ns", None):
            _LAST_EXEC_NS = res.exec_time_ns
    else:
        res = run_bass_kernel_spmd(nc, in_maps, list(range(8)))
        _LAST_RES = res
    ys = [res.results[r]["y_out"] for r in range(8)]
    y = np.concatenate(ys, axis=0).astype(np.float32)
    return y.reshape(B, T, _D)


# revision 60
# speedup vs baseline: 1.1444x; 1.0635x over previous
"""TRN2 Bass kernel for nn_MoEPositionwiseFFN: kernel(**inputs) -> np.ndarray.

v2: expert-parallel MoE FFN without dynamic-DMA dispatch scatter.

Per core r (= expert r):
  P1  router on own 2048 tokens (fp32), AllGather 4 routing planes.
  P2  replicated capacity selection (threshold bisection) -> keep/pos.
  P2.5 own-token extraction; slot->token dispatch table built with
       one-hot rank-1 matmuls into PSUM; ReduceScatter(add) routes each
       expert its [CAPP] slice. Combine indices built via PE transposes.
  P3  expert FFN on 2560 gathered rows (row dma_gather + PE transpose),
       output AllGather chunked (5x) to overlap with compute.
  P4  combine: gather 2 rows/token from all_out, gate-weighted sum
       (gates applied combine-side; ZSLOT row zeroed explicitly).
"""

import os
import sys

for _p in ("/opt/trn_rl_repo", "/opt/pypackages"):
    if _p not in sys.path:
        sys.path.insert(0, _p)


from dataclasses import dataclass

import numpy as np

import concourse.bass as bass
import concourse.bacc as bacc
import concourse.tile as tile
import concourse.mybir as mybir
from concourse.masks import make_identity

FP32 = mybir.dt.float32
BF16 = mybir.dt.bfloat16
I32 = mybir.dt.int32
I16 = mybir.dt.int16
U16 = mybir.dt.uint16
AF = mybir.ActivationFunctionType
ALU = mybir.AluOpType
AX = mybir.AxisListType


@dataclass
class Cfg:
    ncores: int = 8
    E: int = 8
    K: int = 2
    D: int = 1024
    H: int = 4096
    TPC: int = 2048          # tokens per core
    cap: int = 2458          # reference capacity
    CAPP: int = 2560         # padded capacity (= NCHUNK*CB)
    CB: int = 512            # FFN chunk / AllGather chunk (tokens)
    NBIS: int = 26           # bisection iterations (covers 0x3C000000..0x3F800000)

    @property
    def N(self):
        return self.ncores * self.TPC

    @property
    def TT(self):
        return self.TPC // 128  # token tiles per core (16)

    @property
    def M(self):
        return self.N // 128    # global token groups (128)

    @property
    def DCH(self):
        return self.D // 128

    @property
    def HCH(self):
        return self.H // 128

    @property
    def NCHUNK(self):
        return self.CAPP // self.CB

    @property
    def FW(self):
        return self.CAPP // 16  # dispatch-table free width (160)


def build(cfg: Cfg, dbg: bool = False):
    E, K, D, H = cfg.E, cfg.K, cfg.D, cfg.H
    TPC, TT, M, N = cfg.TPC, cfg.TT, cfg.M, cfg.N
    DCH, HCH = cfg.DCH, cfg.HCH
    CAP, CAPP, CB, NCHUNK, FW = cfg.cap, cfg.CAPP, cfg.CB, cfg.NCHUNK, cfg.FW
    NC = cfg.ncores
    assert E == NC == 8 and K == 2
    assert CAPP == NCHUNK * CB and CAPP % 16 == 0 and CB % 128 == 0
    NBLK = CB // 128          # 128-token blocks per chunk (4)
    # combine flat row for dropped assignments: chunk layout row of
    # (expert 0, slot CAPP-1), guaranteed zeroed.
    ZC = (CAPP - 1) // CB
    ZOFF = ZC * (NC * CB) + 0 * CB + ((CAPP - 1) - ZC * CB)

    nc = bacc.Bacc("TRN2", target_bir_lowering=False, debug=False,
                   num_devices=NC)

    # ---- external inputs (per-core staged by host) ----
    xT_shard = nc.dram_tensor("xT_shard", [D, TPC], FP32, kind="ExternalInput")
    x_bf16 = nc.dram_tensor("x_bf16", [N, D], BF16, kind="ExternalInput")
    Wr_in = nc.dram_tensor("Wr_in", [128, DCH, E], FP32, kind="ExternalInput")
    br_in = nc.dram_tensor("br_in", [1, E], FP32, kind="ExternalInput")
    W1_in = nc.dram_tensor("W1_in", [128, DCH, H], BF16, kind="ExternalInput")
    W2_in = nc.dram_tensor("W2_in", [128, HCH, D], BF16, kind="ExternalInput")
    b1_in = nc.dram_tensor("b1_in", [128, HCH], FP32, kind="ExternalInput")
    b2_in = nc.dram_tensor("b2_in", [1, D], BF16, kind="ExternalInput")
    ltri_in = nc.dram_tensor("ltri_in", [128, 128], BF16, kind="ExternalInput")
    rk_in = nc.dram_tensor("rk_in", [128, 1], FP32, kind="ExternalInput")

    # ---- external output ----
    y_out = nc.dram_tensor("y_out", [TPC, D], FP32, kind="ExternalOutput")

    if dbg:
        dbg_rta = nc.dram_tensor("dbg_rta", [4, 128, M], FP32,
                                 kind="ExternalOutput")
        dbg_pos = nc.dram_tensor("dbg_pos", [128, E * M], FP32,
                                 kind="ExternalOutput")
        dbg_keep = nc.dram_tensor("dbg_keep", [128, E * M], FP32,
                                  kind="ExternalOutput")
        dbg_tab = nc.dram_tensor("dbg_tab", [16, 2 * FW], FP32,
                                 kind="ExternalOutput")
        dbg_cidx = nc.dram_tensor("dbg_cidx", [128, K * TT * 8], I16,
                                  kind="ExternalOutput")
        dbg_didx = nc.dram_tensor("dbg_didx", [128, FW], I16,
                                  kind="ExternalOutput")
        dbg_oe = nc.dram_tensor("dbg_oe", [CAPP, D], BF16,
                                kind="ExternalOutput")

    with tile.TileContext(nc) as tc:
        rank_sp = nc.partition_id()

        import contextlib
        top = contextlib.ExitStack()
        cpool = top.enter_context(tc.tile_pool(name="const", bufs=1))
        wts = top.enter_context(tc.tile_pool(name="wts", bufs=1))
        keepp = top.enter_context(tc.tile_pool(name="keepp", bufs=1))
        dramp = top.enter_context(tc.tile_pool(name="dramp", bufs=1,
                                               space="DRAM"))

        # ---- DRAM scratch ----
        rt_local = dramp.tile([4, 128, TT], FP32, tag="rt_local")
        rt_all = dramp.tile([NC, 4, 128, TT], FP32, tag="rt_all",
                            addr_space="Shared")
        tab_dram = dramp.tile([128, 2 * FW], FP32, tag="tab_dram")
        tab_all = dramp.tile([NC, 128, 2 * FW], FP32, tag="tab_all",
                             addr_space="Shared")
        out_e = dramp.tile([CAPP, D], BF16, tag="out_e")
        AG_CHUNKED = True
        if AG_CHUNKED:
            all_out = dramp.tile([NCHUNK, NC, CB, D], BF16, tag="all_out")
        else:
            all_out = dramp.tile([NC, CAPP, D], BF16, tag="all_out",
                                 addr_space="Shared")

        # ---- weights: DMA starts immediately, overlaps P1/P2 ----
        W1s = wts.tile([128, DCH, H], BF16, tag="W1s")
        nc.sync.dma_start(W1s, W1_in[:, :, :])
        W2s = wts.tile([128, HCH, D], BF16, tag="W2s")
        nc.sync.dma_start(W2s, W2_in[:, :, :])
        b1s = wts.tile([128, HCH], FP32, tag="b1s")
        nc.sync.dma_start(b1s, b1_in[:, :])
        b2s = wts.tile([1, D], BF16, tag="b2s")
        nc.sync.dma_start(b2s, b2_in[:, :])

        # ---- constants ----
        ident_b = cpool.tile([128, 128], BF16, tag="ident_b")
        make_identity(nc, ident_b)
        ident_f = cpool.tile([128, 128], FP32, tag="ident_f")
        make_identity(nc, ident_f)
        ltri = cpool.tile([128, 128], BF16, tag="ltri")
        nc.sync.dma_start(ltri, ltri_in[:, :])
        wr_sb = cpool.tile([128, DCH, E], FP32, tag="wr")
        nc.sync.dma_start(wr_sb, Wr_in[:, :, :])
        br_sb = cpool.tile([1, E], FP32, tag="br")
        nc.sync.dma_start(br_sb, br_in[:, :])
        rk_sb = cpool.tile([128, 1], FP32, tag="rk_sb")
        nc.sync.dma_start(rk_sb, rk_in[:, :])
        ones1f = cpool.tile([1, 128], FP32, tag="ones1f")
        nc.vector.memset(ones1f, 1.0)
        ones1b = cpool.tile([1, 128], BF16, tag="ones1b")
        nc.vector.memset(ones1b, 1.0)
        ones128b = cpool.tile([128, 128], BF16, tag="ones128b")
        nc.vector.memset(ones128b, 1.0)
        one_i = cpool.tile([128, E], I32, tag="one_i")
        nc.vector.memset(one_i, 1)
        # iota along free: F160[p, j] = j ; F128 = F160[:, :128]
        it_i = cpool.tile([128, FW], I32, tag="it_i")
        nc.gpsimd.iota(it_i, pattern=[[1, FW]], base=0, channel_multiplier=0)
        F160 = cpool.tile([128, FW], FP32, tag="F160")
        nc.vector.tensor_copy(F160, it_i)
        F128 = F160[:, 0:128]
        # lovals[p, 0] = p
        lov_i = cpool.tile([128, 1], I32, tag="lov_i")
        nc.gpsimd.iota(lov_i, pattern=[[1, 1]], base=0, channel_multiplier=1)
        lovals = cpool.tile([128, 1], FP32, tag="lovals")
        nc.vector.tensor_copy(lovals, lov_i)
        # hival[p, t] = rank*TT + t   (token-group id of own tile t)
        tt_i = cpool.tile([128, TT], I32, tag="tt_i")
        nc.gpsimd.iota(tt_i, pattern=[[1, TT]], base=0, channel_multiplier=0)
        hival = cpool.tile([128, TT], FP32, tag="hival")
        nc.vector.tensor_copy(hival, tt_i)
        nc.vector.tensor_scalar(hival, hival, rk_sb, None, op0=ALU.add)
        # zmask[p] = 0 for p == 127 else 1 (ZSLOT row kill)
        zmask = cpool.tile([128, 1], FP32, tag="zmask")
        nc.vector.tensor_scalar(zmask, lovals, 127.0, None,
                                op0=ALU.not_equal)
        # tokv[p, t] = global token id of own (t, p) = (rk+t)*128 + p
        tokv = cpool.tile([128, TT], FP32, tag="tokv")
        nc.vector.tensor_scalar(tokv, hival, 128.0, lovals,
                                op0=ALU.mult, op1=ALU.add)

        # ---- persistent small tiles (survive into P3/P4) ----
        dIdx = keepp.tile([128, FW], I16, tag="dIdx")
        ci16 = keepp.tile([128, K * TT * 8], I16, tag="ci16")
        g1o = keepp.tile([128, TT], FP32, tag="g1o")
        g2o = keepp.tile([128, TT], FP32, tag="g2o")
        gdp = keepp.tile([128, NCHUNK * NBLK], FP32, tag="gdp")

        selstack = contextlib.ExitStack()
        sel = selstack.enter_context(tc.tile_pool(name="sel", bufs=1))
        lrp = selstack.enter_context(tc.tile_pool(name="lrp", bufs=2))
        psr = selstack.enter_context(
            tc.tile_pool(name="psr", bufs=2, space="PSUM"))
        pscnt = selstack.enter_context(
            tc.tile_pool(name="pscnt", bufs=2, space="PSUM"))
        pstab = selstack.enter_context(
            tc.tile_pool(name="pstab", bufs=1, space="PSUM"))
        pstr = selstack.enter_context(
            tc.tile_pool(name="pstr", bufs=1, space="PSUM"))
        xstack = contextlib.ExitStack()
        xpool = xstack.enter_context(tc.tile_pool(name="xpool", bufs=2))

        # ---------- P1: router on own shard ----------
        # logits via Wr-stationary matmuls: ps8[e, tok] = sum_d Wr[d,e]x[d,tok]
        br8 = cpool.tile([8, 1], FP32, tag="br8")
        nc.sync.dma_start(br8, br_in.rearrange("o e -> e o"))
        E_sb = sel.tile([128, TT, E], FP32, tag="E_sb")
        QT = TT // 4  # t-tiles per quarter (512 tokens)
        for q4 in range(4):
            xq = xpool.tile([128, DCH, QT * 128], FP32, tag="xq")
            nc.sync.dma_start(
                xq,
                xT_shard[:, q4 * QT * 128:(q4 + 1) * QT * 128]
                .rearrange("(dch p) t -> p dch t", p=128))
            ps8 = psr.tile([8, QT * 128], FP32, tag="ps8")
            for dch in range(DCH):
                nc.tensor.matmul(ps8, lhsT=wr_sb[:, dch, :],
                                 rhs=xq[:, dch, :],
                                 start=(dch == 0), stop=(dch == DCH - 1))
            sb8 = xpool.tile([8, QT * 128], FP32, tag="sb8")
            nc.scalar.activation(sb8, ps8, AF.Identity, bias=br8)
            for tr in range(QT):
                t = q4 * QT + tr
                psT = pstr.tile([128, 8], FP32, tag="psT8")
                nc.tensor.transpose(psT, sb8[:, tr * 128:(tr + 1) * 128],
                                    ident_f[0:8, 0:8])
                nc.scalar.activation(E_sb[:, t, :], psT, AF.Exp)
        xstack.close()
        selB = selstack.enter_context(tc.tile_pool(name="selB", bufs=1))
        # batched top-2 over the expert axis
        e8i = sel.tile([128, TT * 8], I16, tag="e8i")
        nc.gpsimd.iota(e8i, pattern=[[0, TT], [1, 8]], base=0,
                       channel_multiplier=0)
        e8f = sel.tile([128, TT, 8], FP32, tag="e8f")
        nc.vector.tensor_copy(e8f.rearrange("p a b -> p (a b)"), e8i)
        Z_sb = sel.tile([128, TT], FP32, tag="Z_sb")
        nc.vector.tensor_reduce(Z_sb, E_sb, AX.X, ALU.add)
        m1 = sel.tile([128, TT], FP32, tag="m1")
        nc.vector.tensor_reduce(m1, E_sb, AX.X, ALU.max)
        eqx = sel.tile([128, TT, 8], FP32, tag="eqx")
        nc.vector.tensor_tensor(
            eqx, E_sb, m1.unsqueeze(2).broadcast_to((128, TT, 8)),
            ALU.is_equal)
        tmp8 = sel.tile([128, TT, 8], FP32, tag="tmp8")
        nc.vector.tensor_tensor(tmp8, eqx, e8f, ALU.mult)
        P_i1 = sel.tile([128, TT], FP32, tag="P_i1")
        nc.vector.tensor_reduce(P_i1, tmp8, AX.X, ALU.add)
        nc.vector.tensor_scalar(tmp8, eqx, -1e30, None, op0=ALU.mult)
        nc.vector.tensor_tensor(E_sb, E_sb, tmp8, ALU.add)  # mask out top-1
        m2 = sel.tile([128, TT], FP32, tag="m2")
        nc.vector.tensor_reduce(m2, E_sb, AX.X, ALU.max)
        nc.vector.tensor_tensor(
            eqx, E_sb, m2.unsqueeze(2).broadcast_to((128, TT, 8)),
            ALU.is_equal)
        nc.vector.tensor_tensor(tmp8, eqx, e8f, ALU.mult)
        P_i2 = sel.tile([128, TT], FP32, tag="P_i2")
        nc.vector.tensor_reduce(P_i2, tmp8, AX.X, ALU.add)
        rZ = sel.tile([128, TT], FP32, tag="rZ")
        nc.vector.reciprocal(rZ, Z_sb)
        P_g1 = sel.tile([128, TT], FP32, tag="P_g1")
        P_g2 = sel.tile([128, TT], FP32, tag="P_g2")
        nc.vector.tensor_tensor(P_g1, m1, rZ, ALU.mult)
        nc.vector.tensor_tensor(P_g2, m2, rZ, ALU.mult)
        nc.sync.dma_start(rt_local[0], P_i1)
        nc.sync.dma_start(rt_local[1], P_g1)
        nc.sync.dma_start(rt_local[2], P_i2)
        nc.sync.dma_start(rt_local[3], P_g2)
        nc.gpsimd.collective_compute(
            "AllGather", ALU.bypass,
            replica_groups=[list(range(NC))],
            ins=[rt_local.opt()], outs=[rt_all.opt()])

        # ---------- P2: replicated selection ----------
        i1f = sel.tile([128, M], FP32, tag="i1f")
        g1f = sel.tile([128, M], FP32, tag="g1f")
        i2f = sel.tile([128, M], FP32, tag="i2f")
        g2f = sel.tile([128, M], FP32, tag="g2f")
        for q, dst in ((0, i1f), (1, g1f), (2, i2f), (3, g2f)):
            nc.sync.dma_start(
                dst.rearrange("p (r t) -> p r t", r=NC),
                rt_all[:, q, :, :].rearrange("r p t -> p r t"))
        if dbg:
            nc.sync.dma_start(dbg_rta[0], i1f)
            nc.sync.dma_start(dbg_rta[1], g1f)
            nc.sync.dma_start(dbg_rta[2], i2f)
            nc.sync.dma_start(dbg_rta[3], g2f)

        A_sb = selB.tile([128, E, M], FP32, tag="A_sb")
        tmpM = sel.tile([128, M], FP32, tag="tmpM")
        for e in range(E):
            nc.vector.scalar_tensor_tensor(
                A_sb[:, e, :], i1f, float(e), g1f,
                op0=ALU.is_equal, op1=ALU.mult)
            nc.vector.scalar_tensor_tensor(
                tmpM, i2f, float(e), g2f, op0=ALU.is_equal, op1=ALU.mult)
            nc.vector.tensor_tensor(A_sb[:, e, :], A_sb[:, e, :], tmpM,
                                    ALU.add)

        big = selB.tile([128, E, M], FP32, tag="big")
        cntp = sel.tile([128, E], FP32, tag="cntp")
        cntb = sel.tile([128, E], BF16, tag="cntb")
        cntf = sel.tile([128, E], FP32, tag="cntf")
        Ktgt = sel.tile([128, E], FP32, tag="Ktgt")
        lo = sel.tile([128, E], I32, tag="lo")
        hi = sel.tile([128, E], I32, tag="hi")
        mid = sel.tile([128, E], I32, tag="mid")
        condi = sel.tile([128, E], I32, tag="condi")

        nc.vector.tensor_scalar(big, A_sb, 0.0, None, op0=ALU.is_gt)
        nc.vector.tensor_reduce(cntp, big, AX.X, ALU.add)
        nc.vector.tensor_copy(cntb, cntp)
        pc = pscnt.tile([128, E], FP32, tag="pscnt")
        nc.tensor.matmul(pc, lhsT=ones128b, rhs=cntb, start=True, stop=True)
        nc.vector.tensor_scalar(Ktgt, pc, float(CAP), None, op0=ALU.min)

        zerosM = selB.tile([128, M], FP32, tag="zerosM")
        nc.vector.memset(zerosM, 0.0)
        nc.vector.memset(lo, 0x3C000000)
        nc.vector.memset(hi, 0x3F800000)
        for it in range(cfg.NBIS):
            nc.vector.tensor_tensor(mid, lo, hi, ALU.add)
            nc.vector.tensor_tensor(mid, mid, one_i,
                                    ALU.logical_shift_right)
            midf = mid.bitcast(FP32)
            for e in range(E):
                nc.vector.scalar_tensor_tensor(
                    big[:, e, :], A_sb[:, e, :], midf[:, e:e + 1], zerosM,
                    op0=ALU.is_gt, op1=ALU.add,
                    accum_out=cntp[:, e:e + 1])
            nc.vector.tensor_copy(cntb, cntp)
            pc = pscnt.tile([128, E], FP32, tag="pscnt")
            nc.tensor.matmul(pc, lhsT=ones128b, rhs=cntb, start=True,
                             stop=True)
            nc.vector.tensor_copy(cntf, pc)
            nc.vector.tensor_tensor(condi, cntf, Ktgt, ALU.is_ge)
            nc.vector.copy_predicated(lo, condi, mid)
            nc.vector.tensor_tensor(condi, cntf, Ktgt, ALU.is_lt)
            nc.vector.copy_predicated(hi, condi, mid)

        thrf = lo.bitcast(FP32)
        keepf = selB.tile([128, E, M], FP32, tag="keepf")
        nc.vector.tensor_tensor(
            keepf, A_sb, thrf.unsqueeze(2).broadcast_to((128, E, M)),
            ALU.is_gt)

        rp = selB.tile([128, E, M], FP32, tag="rp")
        for e in range(E):
            nc.vector.tensor_tensor_scan(
                rp[:, e, :], keepf[:, e, :], zerosM, initial=0.0,
                op0=ALU.add, op1=ALU.add)
        totb = sel.tile([128, E], BF16, tag="totb")
        nc.vector.tensor_copy(totb, rp[:, :, M - 1])
        pe_x = pscnt.tile([128, E], FP32, tag="pscnt")
        nc.tensor.matmul(pe_x, lhsT=ltri, rhs=totb, start=True, stop=True)
        excl = sel.tile([128, E], FP32, tag="excl")
        nc.vector.tensor_copy(excl, pe_x)
        pos = selB.tile([128, E, M], FP32, tag="pos")
        nc.vector.tensor_tensor(pos, rp, keepf, ALU.subtract)
        nc.vector.tensor_tensor(
            pos, pos, excl.unsqueeze(2).broadcast_to((128, E, M)),
            ALU.add)
        if dbg:
            nc.sync.dma_start(dbg_pos[:, :],
                              pos.rearrange("p e m -> p (e m)"))
            nc.sync.dma_start(dbg_keep[:, :],
                              keepf.rearrange("p e m -> p (e m)"))

        # ---------- P2.5: own-token extraction + dispatch table ----------
        own0 = bass.ds(rank_sp * TT, TT)
        c15 = sel.tile([128, TT], I32, tag="c15")
        nc.vector.memset(c15, 15)
        c4 = sel.tile([128, TT], I32, tag="c4")
        nc.vector.memset(c4, 4)
        c511 = sel.tile([128, TT], I32, tag="c511")
        nc.vector.memset(c511, 511)
        c9 = sel.tile([128, TT], I32, tag="c9")
        nc.vector.memset(c9, 9)
        pl_k = []      # [128, TT] fp32 per k: table row  e*16 + pos%16
        fs_k = []      # [128, TT] fp32 per k: table col  pos//16 (999=dead)
        offall = sel.tile([128, K * TT], FP32, tag="offall")
        tmpT = sel.tile([128, TT], FP32, tag="tmpT")
        for k in range(K):
            ikf = i1f if k == 0 else i2f
            gkf = g1f if k == 0 else g2f
            go = g1o if k == 0 else g2o
            nc.vector.tensor_copy(go, gkf[:, own0])
            iko = sel.tile([128, TT], FP32, tag=f"iko{k}")
            nc.vector.tensor_copy(iko, ikf[:, own0])
            posk = sel.tile([128, TT], FP32, tag=f"posk{k}")
            keepk = sel.tile([128, TT], FP32, tag=f"keepk{k}")
            first = True
            for e in range(E):
                dst = posk if first else tmpT
                nc.vector.scalar_tensor_tensor(
                    dst, iko, float(e), pos[:, e, own0],
                    op0=ALU.is_equal, op1=ALU.mult)
                if not first:
                    nc.vector.tensor_tensor(posk, posk, tmpT, ALU.add)
                first = False
            first = True
            for e in range(E):
                dst = keepk if first else tmpT
                nc.vector.scalar_tensor_tensor(
                    dst, iko, float(e), keepf[:, e, own0],
                    op0=ALU.is_equal, op1=ALU.mult)
                if not first:
                    nc.vector.tensor_tensor(keepk, keepk, tmpT, ALU.add)
                first = False
            keepi = sel.tile([128, TT], I32, tag=f"keepi{k}")
            nc.vector.tensor_copy(keepi, keepk)
            # table coords (integer split of pos: %16 and //16)
            posI = sel.tile([128, TT], I32, tag=f"posI{k}")
            nc.vector.tensor_copy(posI, posk)
            tmpI = sel.tile([128, TT], I32, tag=f"tmpI{k}")
            nc.vector.tensor_tensor(tmpI, posI, c15, ALU.bitwise_and)
            qo = sel.tile([128, TT], FP32, tag=f"qo{k}")
            nc.vector.tensor_copy(qo, tmpI)
            nc.vector.tensor_tensor(tmpI, posI, c4, ALU.logical_shift_right)
            fo = sel.tile([128, TT], FP32, tag=f"fo{k}")
            nc.vector.tensor_copy(fo, tmpI)
            plo = sel.tile([128, TT], FP32, tag=f"plo{k}")
            nc.vector.scalar_tensor_tensor(
                plo, iko, 16.0, qo, op0=ALU.mult, op1=ALU.add)
            fsel = sel.tile([128, TT], FP32, tag=f"fsel{k}")
            nc.vector.memset(fsel, 999.0)
            nc.vector.copy_predicated(fsel, keepi, fo)
            pl_k.append(plo)
            fs_k.append(fsel)
            off = sel.tile([128, TT], FP32, tag=f"off{k}")
            if AG_CHUNKED:
                # combine flat row (chunk layout): c = pos//CB;
                # off = c*(NC*CB) + ik*CB + pos%CB ; dropped -> ZOFF
                nc.vector.tensor_tensor(tmpI, posI, c511, ALU.bitwise_and)
                m5 = sel.tile([128, TT], FP32, tag=f"m5{k}")
                nc.vector.tensor_copy(m5, tmpI)
                nc.vector.tensor_tensor(tmpI, posI, c9,
                                        ALU.logical_shift_right)
                cdv = sel.tile([128, TT], FP32, tag=f"cdv{k}")
                nc.vector.tensor_copy(cdv, tmpI)
                nc.vector.tensor_scalar(cdv, cdv, float(NC * CB), None,
                                        op0=ALU.mult)
                nc.vector.scalar_tensor_tensor(
                    off, iko, float(CB), m5, op0=ALU.mult, op1=ALU.add)
                nc.vector.tensor_tensor(off, off, cdv, ALU.add)
                zoff = float(ZOFF)
            else:
                # off = ik*CAPP + pos ; dropped -> expert0 slot CAPP-1
                nc.vector.scalar_tensor_tensor(
                    off, iko, float(CAPP), posk, op0=ALU.mult, op1=ALU.add)
                zoff = float(CAPP - 1)
            offd = sel.tile([128, TT], FP32, tag=f"offd{k}")
            nc.vector.memset(offd, zoff)
            nc.vector.copy_predicated(offd, keepi, off)
            nc.vector.tensor_copy(offall[:, k * TT:(k + 1) * TT], offd)

        # table build: psTab[row, :] += sum over items of onehot outer
        # (fp32 planes: [0:FW] token id, [FW:2FW] gate)
        psTab = pstab.tile([128, 2 * FW], FP32, tag="psTab")
        nck = 0
        for k in range(K):
            go = g1o if k == 0 else g2o
            for t in range(TT):
                L = lrp.tile([128, 128], FP32, tag="L")
                nc.vector.tensor_scalar(
                    L, F128, pl_k[k][:, t:t + 1], None, op0=ALU.is_equal)
                R = lrp.tile([128, 2 * FW], FP32, tag="R")
                nc.vector.tensor_scalar(
                    R[:, 0:FW], F160, fs_k[k][:, t:t + 1], tokv[:, t:t + 1],
                    op0=ALU.is_equal, op1=ALU.mult)
                nc.vector.tensor_scalar(
                    R[:, FW:2 * FW], F160, fs_k[k][:, t:t + 1],
                    go[:, t:t + 1], op0=ALU.is_equal, op1=ALU.mult)
                nc.tensor.matmul(psTab, lhsT=L, rhs=R,
                                 start=(nck == 0), stop=(nck == K * TT - 1))
                nck += 1
        tabsb = selB.tile([128, 2 * FW], FP32, tag="tabsb")
        nc.vector.tensor_copy(tabsb, psTab)
        nc.sync.dma_start(tab_dram, tabsb)
        nc.gpsimd.collective_compute(
            "AllGather", ALU.bypass,
            replica_groups=[list(range(NC))],
            ins=[tab_dram.opt()], outs=[tab_all.opt()])

        # readback own expert's 16 rows from each core's table and sum
        own16 = bass.ds(rank_sp * 16, 16)
        tabs = selB.tile([16, 2 * FW], FP32, tag="tabs")
        tabr = selB.tile([16, NC, 2 * FW], FP32, tag="tabr")
        for r in range(NC):
            nc.sync.dma_start(tabr[:, r, :], tab_all[r, own16, :])
        nc.vector.tensor_tensor(tabs, tabr[:, 0, :], tabr[:, 1, :], ALU.add)
        for r in range(2, NC):
            nc.vector.tensor_tensor(tabs, tabs, tabr[:, r, :], ALU.add)
        if dbg:
            nc.sync.dma_start(dbg_tab, tabs)
        dI16 = sel.tile([16, FW], I16, tag="dI16")
        nc.vector.tensor_copy(dI16, tabs[:, 0:FW])
        # per-slot gates, relayout [16q, 160f] -> [128 = (f%8)*16+q, f//8]
        gview = tabs[:, FW:2 * FW].rearrange("q (fd fm) -> q fd fm", fm=8)
        for fm in range(8):
            nc.sync.dma_start(gdp[fm * 16:(fm + 1) * 16, :],
                              gview[:, :, fm])
        for g in range(8):
            nc.sync.dma_start(dIdx[g * 16:(g + 1) * 16, :], dI16)

        # combine idx relayout via PE transposes:
        # cidxf[pl, (k t), ph] = offall[ph*16+pl, (k t)]
        psO = pstr.tile([32, 128], FP32, tag="psO")
        nc.tensor.transpose(psO, offall, ident_f)
        T1 = sel.tile([32, 128], FP32, tag="T1")
        nc.vector.tensor_copy(T1, psO)
        cidxf = sel.tile([128, K * TT, 8], FP32, tag="cidxf")
        for ph in range(8):
            psP = pstr.tile([16, 32], FP32, tag="psP")
            nc.tensor.transpose(psP, T1[:, ph * 16:(ph + 1) * 16],
                                ident_f[0:32, 0:32])
            nc.vector.tensor_copy(cidxf[0:16, :, ph], psP)
        cs16 = sel.tile([16, K * TT * 8], I16, tag="cs16")
        nc.vector.tensor_copy(
            cs16, cidxf[0:16].rearrange("p a b -> p (a b)"))
        for g in range(8):
            nc.sync.dma_start(ci16[g * 16:(g + 1) * 16, :], cs16)
        if dbg:
            nc.sync.dma_start(dbg_cidx, ci16)
            nc.sync.dma_start(dbg_didx, dIdx)

        selstack.close()

        # ---------- P3: expert FFN ----------
        with tc.tile_pool(name="ffn", bufs=2) as ffn, \
             tc.tile_pool(name="htp", bufs=1) as htp, \
             tc.tile_pool(name="ps1", bufs=2, space="PSUM") as ps1p, \
             tc.tile_pool(name="ps2", bufs=2, space="PSUM") as ps2p, \
             tc.tile_pool(name="pst", bufs=2, space="PSUM") as pstp:
            for c in range(NCHUNK):
                xg = ffn.tile([128, NBLK, D], BF16, tag="xg")
                nc.gpsimd.dma_gather(
                    out_ap=xg,
                    in_ap=x_bf16[:, :],
                    idxs_ap=dIdx[:, c * (CB // 16):(c + 1) * (CB // 16)],
                    num_idxs=CB,
                    num_idxs_reg=CB,
                    elem_size=D,
                    transpose=False)
                xTb = ffn.tile([128, DCH, CB], BF16, tag="xTb")
                for dch in range(DCH):
                    psT = pstp.tile([128, CB], BF16, tag="psT")
                    for blk in range(NBLK):
                        nc.tensor.transpose(
                            psT[:, blk * 128:(blk + 1) * 128],
                            xg[:, blk, dch * 128:(dch + 1) * 128],
                            ident_b)
                    nc.vector.tensor_copy(xTb[:, dch, :], psT)
                hT = htp.tile([128, HCH, CB], BF16, tag="hT")
                for j in range(HCH):
                    ps1 = ps1p.tile([128, CB], FP32, tag="ps1")
                    for dch in range(DCH):
                        nc.tensor.matmul(
                            ps1, lhsT=W1s[:, dch, j * 128:(j + 1) * 128],
                            rhs=xTb[:, dch, :],
                            start=(dch == 0), stop=(dch == DCH - 1))
                    sgt = ffn.tile([128, CB], FP32, tag="sgt")
                    nc.scalar.activation(sgt, ps1, AF.Sigmoid,
                                         bias=b1s[:, j:j + 1])
                    nc.vector.scalar_tensor_tensor(
                        hT[:, j, :], ps1, b1s[:, j:j + 1], sgt,
                        op0=ALU.add, op1=ALU.mult)
                for cs in range(NBLK):
                    osb = ffn.tile([128, D], BF16, tag="osb")
                    for dh in range(2):
                        ps2 = ps2p.tile([128, 512], FP32, tag="ps2")
                        for j in range(HCH):
                            nc.tensor.matmul(
                                ps2,
                                lhsT=hT[:, j, cs * 128:(cs + 1) * 128],
                                rhs=W2s[:, j, dh * 512:(dh + 1) * 512],
                                start=(j == 0), stop=False)
                        nc.tensor.matmul(
                            ps2, lhsT=ones1b,
                            rhs=b2s[:, dh * 512:(dh + 1) * 512],
                            start=False, stop=True)
                        blk = c * NBLK + cs
                        nc.vector.tensor_scalar(
                            osb[:, dh * 512:(dh + 1) * 512], ps2,
                            gdp[:, blk:blk + 1], None, op0=ALU.mult)
                    nc.sync.dma_start(
                        out_e[(c * NBLK + cs) * 128:
                              (c * NBLK + cs + 1) * 128, :],
                        osb)
                if AG_CHUNKED:
                    nc.gpsimd.collective_compute(
                        "AllGather", ALU.bypass,
                        replica_groups=[list(range(NC))],
                        ins=[out_e[c * CB:(c + 1) * CB, :]],
                        outs=[all_out[c]])
            if not AG_CHUNKED:
                nc.gpsimd.collective_compute(
                    "AllGather", ALU.bypass,
                    replica_groups=[list(range(NC))],
                    ins=[out_e.opt()], outs=[all_out.opt()])
            if dbg:
                nc.sync.dma_start(dbg_oe[:, :], out_e)

        # ---------- P4: combine own shard ----------
        with tc.tile_pool(name="comb", bufs=2) as comb:
            if AG_CHUNKED:
                allv = all_out.rearrange("n r c d -> (n r c) d")
            else:
                allv = all_out.rearrange("r c d -> (r c) d")
            GC = 4  # t-tiles per gather (512 idxs)
            for t0 in range(0, TT, GC):
                gk = []
                for k in range(K):
                    gkt = comb.tile([128, GC, D], BF16, tag=f"gk{k}")
                    gk.append(gkt)
                    nc.gpsimd.dma_gather(
                        out_ap=gkt,
                        in_ap=allv,
                        idxs_ap=ci16[:, (k * TT + t0) * 8:
                                     (k * TT + t0 + GC) * 8],
                        num_idxs=GC * 128,
                        num_idxs_reg=GC * 128,
                        elem_size=D,
                        transpose=False)
                for tr in range(GC):
                    t = t0 + tr
                    ysb = comb.tile([128, D], FP32, tag="ysb")
                    nc.vector.tensor_tensor(ysb, gk[0][:, tr, :],
                                            gk[1][:, tr, :], ALU.add)
                    nc.sync.dma_start(y_out[t * 128:(t + 1) * 128, :], ysb)

        top.close()

    nc.compile()
    return nc


# ---------------- host-side staging ----------------

def bfloat16_np():
    import ml_dtypes
    return ml_dtypes.bfloat16


def stage_inputs(cfg: Cfg, x, Wr, br, W1, b1, W2, b2):
    """x: [N, D] fp32; returns list of per-core input dicts."""
    E, D, H, TPC, NC = cfg.E, cfg.D, cfg.H, cfg.TPC, cfg.ncores
    DCH, HCH, TT = cfg.DCH, cfg.HCH, cfg.TT
    x = np.ascontiguousarray(x, np.float32)
    x_bf = x.astype(bfloat16_np())
    ltri = np.tril(np.ones((128, 128), np.float32), -1).astype(bfloat16_np())
    in_maps = []
    for r in range(NC):
        shard = x[r * TPC:(r + 1) * TPC]
        m = {
            "xT_shard": np.ascontiguousarray(shard.T),
            "x_bf16": x_bf,
            "Wr_in": np.ascontiguousarray(
                Wr.reshape(DCH, 128, E).transpose(1, 0, 2)).astype(np.float32),
            "br_in": br.reshape(1, E).astype(np.float32),
            "W1_in": np.ascontiguousarray(
                W1[r].reshape(DCH, 128, H).transpose(1, 0, 2)
            ).astype(bfloat16_np()),
            "W2_in": np.ascontiguousarray(
                W2[r].reshape(HCH, 128, D).transpose(1, 0, 2)
            ).astype(bfloat16_np()),
            "b1_in": np.ascontiguousarray(
                b1[r].reshape(HCH, 128).T).astype(np.float32),
            "b2_in": b2[r].reshape(1, D).astype(np.float32).astype(
                bfloat16_np()),
            "ltri_in": ltri,
            "rk_in": np.full((128, 1), r * TT, np.float32),
        }
        in_maps.append(m)
    return in_maps


# ---------------- problem binding ----------------

import math as _math

B, T = 8, 2048
_N = B * T
_D = 1024
_CAP = int(_math.ceil(1.2 * _N / 8))  # 2458

_CACHE = {}


def _get_nc():
    if "nc" not in _CACHE:
        cfg = Cfg(D=_D, H=4096, TPC=_N // 8, cap=_CAP, CAPP=2560, CB=512)
        _CACHE["cfg"] = cfg
        _CACHE["nc"] = build(cfg, dbg=bool(int(os.environ.get("KDBG", "0"))))
    return _CACHE["cfg"], _CACHE["nc"]


TRACE = False
_LAST_EXEC_NS = None
_LAST_RES = None


def kernel(x_btd, Wr, br, W1, b1, W2, b2):
    from concourse.bass_utils import run_bass_kernel_spmd

    global _LAST_EXEC_NS, _LAST_RES
    cfg, nc = _get_nc()
    x = np.ascontiguousarray(np.asarray(x_btd), np.float32).reshape(_N, _D)
    in_maps = stage_inputs(
        cfg, x, np.asarray(Wr), np.asarray(br), np.asarray(W1),
        np.asarray(b1), np.asarray(W2), np.asarray(b2))
    if TRACE:
        import shutil
        tdir = "/root/problem/work/trace"
        shutil.rmtree(tdir, ignore_errors=True)
        os.makedirs(tdir, exist_ok=True)
        tcores = (list(range(8))
                  if os.environ.get("KTRACE_ALL", "0") == "1" else [0])
        res = run_bass_kernel_spmd(nc, in_maps, list(range(8)), trace=True,
                                   trace_cores=tcores, tmpdir=tdir)
        _LAST_RES = res
        if getattr(res, "exec_time_ns", None):
            _LAST_EXEC_NS = res.exec_time_ns
    else:
        res = run_bass_kernel_spmd(nc, in_maps, list(range(8)))
        _LAST_RES = res
    ys = [res.results[r]["y_out"] for r in range(8)]
    y = np.concatenate(ys, axis=0).astype(np.float32)
    return y.reshape(B, T, _D)


# revision 63
# speedup vs baseline: 1.1852x; 1.0357x over previous
"""TRN2 Bass kernel for nn_MoEPositionwiseFFN: kernel(**inputs) -> np.ndarray.

v2: expert-parallel MoE FFN without dynamic-DMA dispatch scatter.

Per core r (= expert r):
  P1  router on own 2048 tokens (fp32), AllGather 4 routing planes.
  P2  replicated capacity selection (threshold bisection) -> keep/pos.
  P2.5 own-token extraction; slot->token dispatch table built with
       one-hot rank-1 matmuls into PSUM; ReduceScatter(add) routes each
       expert its [CAPP] slice. Combine indices built via PE transposes.
  P3  expert FFN on 2560 gathered rows (row dma_gather + PE transpose),
       output AllGather chunked (5x) to overlap with compute.
  P4  combine: gather 2 rows/token from all_out, gate-weighted sum
       (gates applied combine-side; ZSLOT row zeroed explicitly).
"""

import os
import sys

for _p in ("/opt/trn_rl_repo", "/opt/pypackages"):
    if _p not in sys.path:
        sys.path.insert(0, _p)


from dataclasses import dataclass

import numpy as np

import concourse.bass as bass
import concourse.bacc as bacc
import concourse.tile as tile
import concourse.mybir as mybir
from concourse.masks import make_identity

FP32 = mybir.dt.float32
BF16 = mybir.dt.bfloat16
I32 = mybir.dt.int32
I16 = mybir.dt.int16
U16 = mybir.dt.uint16
AF = mybir.ActivationFunctionType
ALU = mybir.AluOpType
AX = mybir.AxisListType


@dataclass
class Cfg:
    ncores: int = 8
    E: int = 8
    K: int = 2
    D: int = 1024
    H: int = 4096
    TPC: int = 2048          # tokens per core
    cap: int = 2458          # reference capacity
    CAPP: int = 2560         # padded capacity (= NCHUNK*CB)
    CB: int = 512            # FFN chunk / AllGather chunk (tokens)
    NBIS: int = 26           # bisection iterations (covers 0x3C000000..0x3F800000)

    @property
    def N(self):
        return self.ncores * self.TPC

    @property
    def TT(self):
        return self.TPC // 128  # token tiles per core (16)

    @property
    def M(self):
        return self.N // 128    # global token groups (128)

    @property
    def DCH(self):
        return self.D // 128

    @property
    def HCH(self):
        return self.H // 128

    @property
    def NCHUNK(self):
        return self.CAPP // self.CB

    @property
    def FW(self):
        return self.CAPP // 16  # dispatch-table free width (160)


def build(cfg: Cfg, dbg: bool = False):
    E, K, D, H = cfg.E, cfg.K, cfg.D, cfg.H
    TPC, TT, M, N = cfg.TPC, cfg.TT, cfg.M, cfg.N
    DCH, HCH = cfg.DCH, cfg.HCH
    CAP, CAPP, CB, NCHUNK, FW = cfg.cap, cfg.CAPP, cfg.CB, cfg.NCHUNK, cfg.FW
    NC = cfg.ncores
    assert E == NC == 8 and K == 2
    assert CAPP == NCHUNK * CB and CAPP % 16 == 0 and CB % 128 == 0
    NBLK = CB // 128          # 128-token blocks per chunk (4)
    # combine flat row for dropped assignments: chunk layout row of
    # (expert 0, slot CAPP-1), guaranteed zeroed.
    ZC = (CAPP - 1) // CB
    ZOFF = ZC * (NC * CB) + 0 * CB + ((CAPP - 1) - ZC * CB)

    nc = bacc.Bacc("TRN2", target_bir_lowering=False, debug=False,
                   num_devices=NC)

    # ---- external inputs (per-core staged by host) ----
    xT_shard = nc.dram_tensor("xT_shard", [D, TPC], FP32, kind="ExternalInput")
    x_bf16 = nc.dram_tensor("x_bf16", [N, D], BF16, kind="ExternalInput")
    Wr_in = nc.dram_tensor("Wr_in", [128, DCH, E], FP32, kind="ExternalInput")
    br_in = nc.dram_tensor("br_in", [1, E], FP32, kind="ExternalInput")
    W1_in = nc.dram_tensor("W1_in", [128, DCH, H], BF16, kind="ExternalInput")
    W2_in = nc.dram_tensor("W2_in", [128, HCH, D], BF16, kind="ExternalInput")
    b1_in = nc.dram_tensor("b1_in", [128, HCH], FP32, kind="ExternalInput")
    b2_in = nc.dram_tensor("b2_in", [1, D], BF16, kind="ExternalInput")
    ltri_in = nc.dram_tensor("ltri_in", [128, 128], BF16, kind="ExternalInput")
    rk_in = nc.dram_tensor("rk_in", [128, 1], FP32, kind="ExternalInput")

    # ---- external output ----
    y_out = nc.dram_tensor("y_out", [TPC, D], FP32, kind="ExternalOutput")

    if dbg:
        dbg_rta = nc.dram_tensor("dbg_rta", [4, 128, M], FP32,
                                 kind="ExternalOutput")
        dbg_pos = nc.dram_tensor("dbg_pos", [128, E * M], FP32,
                                 kind="ExternalOutput")
        dbg_keep = nc.dram_tensor("dbg_keep", [128, E * M], FP32,
                                  kind="ExternalOutput")
        dbg_tab = nc.dram_tensor("dbg_tab", [16, 2 * FW], FP32,
                                 kind="ExternalOutput")
        dbg_cidx = nc.dram_tensor("dbg_cidx", [128, K * TT * 8], I16,
                                  kind="ExternalOutput")
        dbg_didx = nc.dram_tensor("dbg_didx", [128, FW], I16,
                                  kind="ExternalOutput")
        dbg_oe = nc.dram_tensor("dbg_oe", [CAPP, D], BF16,
                                kind="ExternalOutput")

    with tile.TileContext(nc) as tc:
        rank_sp = nc.partition_id()

        import contextlib
        top = contextlib.ExitStack()
        cpool = top.enter_context(tc.tile_pool(name="const", bufs=1))
        wts = top.enter_context(tc.tile_pool(name="wts", bufs=1))
        keepp = top.enter_context(tc.tile_pool(name="keepp", bufs=1))
        dramp = top.enter_context(tc.tile_pool(name="dramp", bufs=1,
                                               space="DRAM"))

        # ---- DRAM scratch ----
        rt_local = dramp.tile([4, 128, TT], FP32, tag="rt_local")
        rt_all = dramp.tile([NC, 4, 128, TT], FP32, tag="rt_all",
                            addr_space="Shared")
        tab_dram = dramp.tile([128, 2 * FW], FP32, tag="tab_dram")
        tab_all = dramp.tile([NC, 128, 2 * FW], FP32, tag="tab_all",
                             addr_space="Shared")
        out_e = dramp.tile([CAPP, D], BF16, tag="out_e")
        AG_CHUNKED = True
        if AG_CHUNKED:
            all_out = dramp.tile([NCHUNK, NC, CB, D], BF16, tag="all_out")
        else:
            all_out = dramp.tile([NC, CAPP, D], BF16, tag="all_out",
                                 addr_space="Shared")

        # ---- weights (DMA issued after router loads; overlaps P2) ----
        W1s = wts.tile([128, DCH, H], BF16, tag="W1s")
        W2s = wts.tile([128, HCH, D], BF16, tag="W2s")
        b1s = wts.tile([128, HCH], FP32, tag="b1s")
        b2s = wts.tile([1, D], BF16, tag="b2s")

        # ---- constants ----
        ident_b = cpool.tile([128, 128], BF16, tag="ident_b")
        make_identity(nc, ident_b)
        ident_f = cpool.tile([128, 128], FP32, tag="ident_f")
        make_identity(nc, ident_f)
        ltri = cpool.tile([128, 128], BF16, tag="ltri")
        nc.sync.dma_start(ltri, ltri_in[:, :])
        wr_sb = cpool.tile([128, DCH, E], FP32, tag="wr")
        nc.sync.dma_start(wr_sb, Wr_in[:, :, :])
        br_sb = cpool.tile([1, E], FP32, tag="br")
        nc.sync.dma_start(br_sb, br_in[:, :])
        rk_sb = cpool.tile([128, 1], FP32, tag="rk_sb")
        nc.sync.dma_start(rk_sb, rk_in[:, :])
        ones1f = cpool.tile([1, 128], FP32, tag="ones1f")
        nc.vector.memset(ones1f, 1.0)
        ones1b = cpool.tile([1, 128], BF16, tag="ones1b")
        nc.vector.memset(ones1b, 1.0)
        ones128b = cpool.tile([128, 128], BF16, tag="ones128b")
        nc.vector.memset(ones128b, 1.0)
        one_i = cpool.tile([128, E], I32, tag="one_i")
        nc.vector.memset(one_i, 1)
        # iota along free: F160[p, j] = j ; F128 = F160[:, :128]
        it_i = cpool.tile([128, FW], I32, tag="it_i")
        nc.gpsimd.iota(it_i, pattern=[[1, FW]], base=0, channel_multiplier=0)
        F160 = cpool.tile([128, FW], FP32, tag="F160")
        nc.vector.tensor_copy(F160, it_i)
        F128 = F160[:, 0:128]
        # lovals[p, 0] = p
        lov_i = cpool.tile([128, 1], I32, tag="lov_i")
        nc.gpsimd.iota(lov_i, pattern=[[1, 1]], base=0, channel_multiplier=1)
        lovals = cpool.tile([128, 1], FP32, tag="lovals")
        nc.vector.tensor_copy(lovals, lov_i)
        # hival[p, t] = rank*TT + t   (token-group id of own tile t)
        tt_i = cpool.tile([128, TT], I32, tag="tt_i")
        nc.gpsimd.iota(tt_i, pattern=[[1, TT]], base=0, channel_multiplier=0)
        hival = cpool.tile([128, TT], FP32, tag="hival")
        nc.vector.tensor_copy(hival, tt_i)
        nc.vector.tensor_scalar(hival, hival, rk_sb, None, op0=ALU.add)
        # zmask[p] = 0 for p == 127 else 1 (ZSLOT row kill)
        zmask = cpool.tile([128, 1], FP32, tag="zmask")
        nc.vector.tensor_scalar(zmask, lovals, 127.0, None,
                                op0=ALU.not_equal)
        # tokv[p, t] = global token id of own (t, p) = (rk+t)*128 + p
        tokv = cpool.tile([128, TT], FP32, tag="tokv")
        nc.vector.tensor_scalar(tokv, hival, 128.0, lovals,
                                op0=ALU.mult, op1=ALU.add)

        # ---- persistent small tiles (survive into P3/P4) ----
        dIdx = keepp.tile([128, FW], I16, tag="dIdx")
        ci16 = keepp.tile([128, K * TT * 8], I16, tag="ci16")
        g1o = keepp.tile([128, TT], FP32, tag="g1o")
        g2o = keepp.tile([128, TT], FP32, tag="g2o")
        gdp = keepp.tile([128, NCHUNK * NBLK], FP32, tag="gdp")

        selstack = contextlib.ExitStack()
        sel = selstack.enter_context(tc.tile_pool(name="sel", bufs=1))
        lrp = selstack.enter_context(tc.tile_pool(name="lrp", bufs=2))
        psr = selstack.enter_context(
            tc.tile_pool(name="psr", bufs=2, space="PSUM"))
        pscnt = selstack.enter_context(
            tc.tile_pool(name="pscnt", bufs=2, space="PSUM"))
        pstab = selstack.enter_context(
            tc.tile_pool(name="pstab", bufs=1, space="PSUM"))
        pstr = selstack.enter_context(
            tc.tile_pool(name="pstr", bufs=1, space="PSUM"))
        xstack = contextlib.ExitStack()
        xpool = xstack.enter_context(tc.tile_pool(name="xpool", bufs=2))

        # ---------- P1: router on own shard ----------
        # logits via Wr-stationary matmuls: ps8[e, tok] = sum_d Wr[d,e]x[d,tok]
        br8 = cpool.tile([8, 1], FP32, tag="br8")
        nc.sync.dma_start(br8, br_in.rearrange("o e -> e o"))
        E_sb = sel.tile([128, TT, E], FP32, tag="E_sb")
        QT = TT // 4  # t-tiles per quarter (512 tokens)
        for q4 in range(4):
            xq = xpool.tile([128, DCH, QT * 128], FP32, tag="xq")
            nc.sync.dma_start(
                xq,
                xT_shard[:, q4 * QT * 128:(q4 + 1) * QT * 128]
                .rearrange("(dch p) t -> p dch t", p=128))
            ps8 = psr.tile([8, QT * 128], FP32, tag="ps8")
            for dch in range(DCH):
                nc.tensor.matmul(ps8, lhsT=wr_sb[:, dch, :],
                                 rhs=xq[:, dch, :],
                                 start=(dch == 0), stop=(dch == DCH - 1))
            sb8 = xpool.tile([8, QT * 128], FP32, tag="sb8")
            nc.scalar.activation(sb8, ps8, AF.Identity, bias=br8)
            for tr in range(QT):
                t = q4 * QT + tr
                psT = pstr.tile([128, 8], FP32, tag="psT8")
                nc.tensor.transpose(psT, sb8[:, tr * 128:(tr + 1) * 128],
                                    ident_f[0:8, 0:8])
                nc.scalar.activation(E_sb[:, t, :], psT, AF.Exp)
        xstack.close()
        selB = selstack.enter_context(tc.tile_pool(name="selB", bufs=1))
        # batched top-2 over the expert axis
        e8i = sel.tile([128, TT * 8], I16, tag="e8i")
        nc.gpsimd.iota(e8i, pattern=[[0, TT], [1, 8]], base=0,
                       channel_multiplier=0)
        e8f = sel.tile([128, TT, 8], FP32, tag="e8f")
        nc.vector.tensor_copy(e8f.rearrange("p a b -> p (a b)"), e8i)
        Z_sb = sel.tile([128, TT], FP32, tag="Z_sb")
        nc.vector.tensor_reduce(Z_sb, E_sb, AX.X, ALU.add)
        m1 = sel.tile([128, TT], FP32, tag="m1")
        nc.vector.tensor_reduce(m1, E_sb, AX.X, ALU.max)
        eqx = sel.tile([128, TT, 8], FP32, tag="eqx")
        nc.vector.tensor_tensor(
            eqx, E_sb, m1.unsqueeze(2).broadcast_to((128, TT, 8)),
            ALU.is_equal)
        tmp8 = sel.tile([128, TT, 8], FP32, tag="tmp8")
        nc.vector.tensor_tensor(tmp8, eqx, e8f, ALU.mult)
        P_i1 = sel.tile([128, TT], FP32, tag="P_i1")
        nc.vector.tensor_reduce(P_i1, tmp8, AX.X, ALU.add)
        nc.vector.tensor_scalar(tmp8, eqx, -1e30, None, op0=ALU.mult)
        nc.vector.tensor_tensor(E_sb, E_sb, tmp8, ALU.add)  # mask out top-1
        m2 = sel.tile([128, TT], FP32, tag="m2")
        nc.vector.tensor_reduce(m2, E_sb, AX.X, ALU.max)
        nc.vector.tensor_tensor(
            eqx, E_sb, m2.unsqueeze(2).broadcast_to((128, TT, 8)),
            ALU.is_equal)
        nc.vector.tensor_tensor(tmp8, eqx, e8f, ALU.mult)
        P_i2 = sel.tile([128, TT], FP32, tag="P_i2")
        nc.vector.tensor_reduce(P_i2, tmp8, AX.X, ALU.add)
        rZ = sel.tile([128, TT], FP32, tag="rZ")
        nc.vector.reciprocal(rZ, Z_sb)
        P_g1 = sel.tile([128, TT], FP32, tag="P_g1")
        P_g2 = sel.tile([128, TT], FP32, tag="P_g2")
        nc.vector.tensor_tensor(P_g1, m1, rZ, ALU.mult)
        nc.vector.tensor_tensor(P_g2, m2, rZ, ALU.mult)
        nc.sync.dma_start(rt_local[0], P_i1)
        nc.sync.dma_start(rt_local[1], P_g1)
        nc.sync.dma_start(rt_local[2], P_i2)
        nc.sync.dma_start(rt_local[3], P_g2)
        nc.gpsimd.collective_compute(
            "AllGather", ALU.bypass,
            replica_groups=[list(range(NC))],
            ins=[rt_local.opt()], outs=[rt_all.opt()])
        nc.sync.dma_start(W1s, W1_in[:, :, :])
        nc.sync.dma_start(W2s, W2_in[:, :, :])
        nc.sync.dma_start(b1s, b1_in[:, :])
        nc.sync.dma_start(b2s, b2_in[:, :])

        # ---------- P2: replicated selection ----------
        i1f = sel.tile([128, M], FP32, tag="i1f")
        g1f = sel.tile([128, M], FP32, tag="g1f")
        i2f = sel.tile([128, M], FP32, tag="i2f")
        g2f = sel.tile([128, M], FP32, tag="g2f")
        for q, dst in ((0, i1f), (1, g1f), (2, i2f), (3, g2f)):
            nc.sync.dma_start(
                dst.rearrange("p (r t) -> p r t", r=NC),
                rt_all[:, q, :, :].rearrange("r p t -> p r t"))
        if dbg:
            nc.sync.dma_start(dbg_rta[0], i1f)
            nc.sync.dma_start(dbg_rta[1], g1f)
            nc.sync.dma_start(dbg_rta[2], i2f)
            nc.sync.dma_start(dbg_rta[3], g2f)

        A_sb = selB.tile([128, E, M], FP32, tag="A_sb")
        tmpM = sel.tile([128, M], FP32, tag="tmpM")
        for e in range(E):
            nc.vector.scalar_tensor_tensor(
                A_sb[:, e, :], i1f, float(e), g1f,
                op0=ALU.is_equal, op1=ALU.mult)
            nc.vector.scalar_tensor_tensor(
                tmpM, i2f, float(e), g2f, op0=ALU.is_equal, op1=ALU.mult)
            nc.vector.tensor_tensor(A_sb[:, e, :], A_sb[:, e, :], tmpM,
                                    ALU.add)

        big = selB.tile([128, E, M], FP32, tag="big")
        cntp = sel.tile([128, E], FP32, tag="cntp")
        cntb = sel.tile([128, E], BF16, tag="cntb")
        cntf = sel.tile([128, E], FP32, tag="cntf")
        Ktgt = sel.tile([128, E], FP32, tag="Ktgt")
        lo = sel.tile([128, E], I32, tag="lo")
        hi = sel.tile([128, E], I32, tag="hi")
        mid = sel.tile([128, E], I32, tag="mid")
        condi = sel.tile([128, E], I32, tag="condi")

        nc.vector.tensor_scalar(big, A_sb, 0.0, None, op0=ALU.is_gt)
        nc.vector.tensor_reduce(cntp, big, AX.X, ALU.add)
        nc.vector.tensor_copy(cntb, cntp)
        pc = pscnt.tile([128, E], FP32, tag="pscnt")
        nc.tensor.matmul(pc, lhsT=ones128b, rhs=cntb, start=True, stop=True)
        nc.vector.tensor_scalar(Ktgt, pc, float(CAP), None, op0=ALU.min)

        zerosM = selB.tile([128, M], FP32, tag="zerosM")
        nc.vector.memset(zerosM, 0.0)
        nc.vector.memset(lo, 0x3C000000)
        nc.vector.memset(hi, 0x3F800000)
        for it in range(cfg.NBIS):
            nc.vector.tensor_tensor(mid, lo, hi, ALU.add)
            nc.vector.tensor_tensor(mid, mid, one_i,
                                    ALU.logical_shift_right)
            midf = mid.bitcast(FP32)
            for e in range(E):
                nc.vector.scalar_tensor_tensor(
                    big[:, e, :], A_sb[:, e, :], midf[:, e:e + 1], zerosM,
                    op0=ALU.is_gt, op1=ALU.add,
                    accum_out=cntp[:, e:e + 1])
            nc.vector.tensor_copy(cntb, cntp)
            pc = pscnt.tile([128, E], FP32, tag="pscnt")
            nc.tensor.matmul(pc, lhsT=ones128b, rhs=cntb, start=True,
                             stop=True)
            nc.vector.tensor_copy(cntf, pc)
            nc.vector.tensor_tensor(condi, cntf, Ktgt, ALU.is_ge)
            nc.vector.copy_predicated(lo, condi, mid)
            nc.vector.tensor_tensor(condi, cntf, Ktgt, ALU.is_lt)
            nc.vector.copy_predicated(hi, condi, mid)

        thrf = lo.bitcast(FP32)
        keepf = selB.tile([128, E, M], FP32, tag="keepf")
        nc.vector.tensor_tensor(
            keepf, A_sb, thrf.unsqueeze(2).broadcast_to((128, E, M)),
            ALU.is_gt)

        rp = selB.tile([128, E, M], FP32, tag="rp")
        for e in range(E):
            nc.vector.tensor_tensor_scan(
                rp[:, e, :], keepf[:, e, :], zerosM, initial=0.0,
                op0=ALU.add, op1=ALU.add)
        totb = sel.tile([128, E], BF16, tag="totb")
        nc.vector.tensor_copy(totb, rp[:, :, M - 1])
        pe_x = pscnt.tile([128, E], FP32, tag="pscnt")
        nc.tensor.matmul(pe_x, lhsT=ltri, rhs=totb, start=True, stop=True)
        excl = sel.tile([128, E], FP32, tag="excl")
        nc.vector.tensor_copy(excl, pe_x)
        pos = selB.tile([128, E, M], FP32, tag="pos")
        nc.vector.tensor_tensor(pos, rp, keepf, ALU.subtract)
        nc.vector.tensor_tensor(
            pos, pos, excl.unsqueeze(2).broadcast_to((128, E, M)),
            ALU.add)
        if dbg:
            nc.sync.dma_start(dbg_pos[:, :],
                              pos.rearrange("p e m -> p (e m)"))
            nc.sync.dma_start(dbg_keep[:, :],
                              keepf.rearrange("p e m -> p (e m)"))

        # ---------- P2.5: own-token extraction + dispatch table ----------
        own0 = bass.ds(rank_sp * TT, TT)
        c15 = sel.tile([128, TT], I32, tag="c15")
        nc.vector.memset(c15, 15)
        c4 = sel.tile([128, TT], I32, tag="c4")
        nc.vector.memset(c4, 4)
        c511 = sel.tile([128, TT], I32, tag="c511")
        nc.vector.memset(c511, 511)
        c9 = sel.tile([128, TT], I32, tag="c9")
        nc.vector.memset(c9, 9)
        pl_k = []      # [128, TT] fp32 per k: table row  e*16 + pos%16
        fs_k = []      # [128, TT] fp32 per k: table col  pos//16 (999=dead)
        offall = sel.tile([128, K * TT], FP32, tag="offall")
        tmpT = sel.tile([128, TT], FP32, tag="tmpT")
        for k in range(K):
            ikf = i1f if k == 0 else i2f
            gkf = g1f if k == 0 else g2f
            go = g1o if k == 0 else g2o
            nc.vector.tensor_copy(go, gkf[:, own0])
            iko = sel.tile([128, TT], FP32, tag=f"iko{k}")
            nc.vector.tensor_copy(iko, ikf[:, own0])
            posk = sel.tile([128, TT], FP32, tag=f"posk{k}")
            keepk = sel.tile([128, TT], FP32, tag=f"keepk{k}")
            first = True
            for e in range(E):
                dst = posk if first else tmpT
                nc.vector.scalar_tensor_tensor(
                    dst, iko, float(e), pos[:, e, own0],
                    op0=ALU.is_equal, op1=ALU.mult)
                if not first:
                    nc.vector.tensor_tensor(posk, posk, tmpT, ALU.add)
                first = False
            first = True
            for e in range(E):
                dst = keepk if first else tmpT
                nc.vector.scalar_tensor_tensor(
                    dst, iko, float(e), keepf[:, e, own0],
                    op0=ALU.is_equal, op1=ALU.mult)
                if not first:
                    nc.vector.tensor_tensor(keepk, keepk, tmpT, ALU.add)
                first = False
            keepi = sel.tile([128, TT], I32, tag=f"keepi{k}")
            nc.vector.tensor_copy(keepi, keepk)
            # table coords (integer split of pos: %16 and //16)
            posI = sel.tile([128, TT], I32, tag=f"posI{k}")
            nc.vector.tensor_copy(posI, posk)
            tmpI = sel.tile([128, TT], I32, tag=f"tmpI{k}")
            nc.vector.tensor_tensor(tmpI, posI, c15, ALU.bitwise_and)
            qo = sel.tile([128, TT], FP32, tag=f"qo{k}")
            nc.vector.tensor_copy(qo, tmpI)
            nc.vector.tensor_tensor(tmpI, posI, c4, ALU.logical_shift_right)
            fo = sel.tile([128, TT], FP32, tag=f"fo{k}")
            nc.vector.tensor_copy(fo, tmpI)
            plo = sel.tile([128, TT], FP32, tag=f"plo{k}")
            nc.vector.scalar_tensor_tensor(
                plo, iko, 16.0, qo, op0=ALU.mult, op1=ALU.add)
            fsel = sel.tile([128, TT], FP32, tag=f"fsel{k}")
            nc.vector.memset(fsel, 999.0)
            nc.vector.copy_predicated(fsel, keepi, fo)
            pl_k.append(plo)
            fs_k.append(fsel)
            off = sel.tile([128, TT], FP32, tag=f"off{k}")
            if AG_CHUNKED:
                # combine flat row (chunk layout): c = pos//CB;
                # off = c*(NC*CB) + ik*CB + pos%CB ; dropped -> ZOFF
                nc.vector.tensor_tensor(tmpI, posI, c511, ALU.bitwise_and)
                m5 = sel.tile([128, TT], FP32, tag=f"m5{k}")
                nc.vector.tensor_copy(m5, tmpI)
                nc.vector.tensor_tensor(tmpI, posI, c9,
                                        ALU.logical_shift_right)
                cdv = sel.tile([128, TT], FP32, tag=f"cdv{k}")
                nc.vector.tensor_copy(cdv, tmpI)
                nc.vector.tensor_scalar(cdv, cdv, float(NC * CB), None,
                                        op0=ALU.mult)
                nc.vector.scalar_tensor_tensor(
                    off, iko, float(CB), m5, op0=ALU.mult, op1=ALU.add)
                nc.vector.tensor_tensor(off, off, cdv, ALU.add)
                zoff = float(ZOFF)
            else:
                # off = ik*CAPP + pos ; dropped -> expert0 slot CAPP-1
                nc.vector.scalar_tensor_tensor(
                    off, iko, float(CAPP), posk, op0=ALU.mult, op1=ALU.add)
                zoff = float(CAPP - 1)
            offd = sel.tile([128, TT], FP32, tag=f"offd{k}")
            nc.vector.memset(offd, zoff)
            nc.vector.copy_predicated(offd, keepi, off)
            nc.vector.tensor_copy(offall[:, k * TT:(k + 1) * TT], offd)

        # table build: psTab[row, :] += sum over items of onehot outer
        # (fp32 planes: [0:FW] token id, [FW:2FW] gate)
        psTab = pstab.tile([128, 2 * FW], FP32, tag="psTab")
        nck = 0
        for k in range(K):
            go = g1o if k == 0 else g2o
            for t in range(TT):
                L = lrp.tile([128, 128], FP32, tag="L")
                nc.vector.tensor_scalar(
                    L, F128, pl_k[k][:, t:t + 1], None, op0=ALU.is_equal)
                R = lrp.tile([128, 2 * FW], FP32, tag="R")
                nc.vector.tensor_scalar(
                    R[:, 0:FW], F160, fs_k[k][:, t:t + 1], tokv[:, t:t + 1],
                    op0=ALU.is_equal, op1=ALU.mult)
                nc.vector.tensor_scalar(
                    R[:, FW:2 * FW], F160, fs_k[k][:, t:t + 1],
                    go[:, t:t + 1], op0=ALU.is_equal, op1=ALU.mult)
                nc.tensor.matmul(psTab, lhsT=L, rhs=R,
                                 start=(nck == 0), stop=(nck == K * TT - 1))
                nck += 1
        tabsb = selB.tile([128, 2 * FW], FP32, tag="tabsb")
        nc.vector.tensor_copy(tabsb, psTab)
        nc.sync.dma_start(tab_dram, tabsb)
        nc.gpsimd.collective_compute(
            "AllGather", ALU.bypass,
            replica_groups=[list(range(NC))],
            ins=[tab_dram.opt()], outs=[tab_all.opt()])

        # readback own expert's 16 rows from each core's table and sum
        own16 = bass.ds(rank_sp * 16, 16)
        tabs = selB.tile([16, 2 * FW], FP32, tag="tabs")
        tabr = selB.tile([16, NC, 2 * FW], FP32, tag="tabr")
        for r in range(NC):
            nc.sync.dma_start(tabr[:, r, :], tab_all[r, own16, :])
        nc.vector.tensor_tensor(tabs, tabr[:, 0, :], tabr[:, 1, :], ALU.add)
        for r in range(2, NC):
            nc.vector.tensor_tensor(tabs, tabs, tabr[:, r, :], ALU.add)
        if dbg:
            nc.sync.dma_start(dbg_tab, tabs)
        dI16 = sel.tile([16, FW], I16, tag="dI16")
        nc.vector.tensor_copy(dI16, tabs[:, 0:FW])
        # per-slot gates, relayout [16q, 160f] -> [128 = (f%8)*16+q, f//8]
        gview = tabs[:, FW:2 * FW].rearrange("q (fd fm) -> q fd fm", fm=8)
        for fm in range(8):
            nc.sync.dma_start(gdp[fm * 16:(fm + 1) * 16, :],
                              gview[:, :, fm])
        for g in range(8):
            nc.sync.dma_start(dIdx[g * 16:(g + 1) * 16, :], dI16)

        # combine idx relayout via PE transposes:
        # cidxf[pl, (k t), ph] = offall[ph*16+pl, (k t)]
        psO = pstr.tile([32, 128], FP32, tag="psO")
        nc.tensor.transpose(psO, offall, ident_f)
        T1 = sel.tile([32, 128], FP32, tag="T1")
        nc.vector.tensor_copy(T1, psO)
        cidxf = sel.tile([128, K * TT, 8], FP32, tag="cidxf")
        for ph in range(8):
            psP = pstr.tile([16, 32], FP32, tag="psP")
            nc.tensor.transpose(psP, T1[:, ph * 16:(ph + 1) * 16],
                                ident_f[0:32, 0:32])
            nc.vector.tensor_copy(cidxf[0:16, :, ph], psP)
        cs16 = sel.tile([16, K * TT * 8], I16, tag="cs16")
        nc.vector.tensor_copy(
            cs16, cidxf[0:16].rearrange("p a b -> p (a b)"))
        for g in range(8):
            nc.sync.dma_start(ci16[g * 16:(g + 1) * 16, :], cs16)
        if dbg:
            nc.sync.dma_start(dbg_cidx, ci16)
            nc.sync.dma_start(dbg_didx, dIdx)

        selstack.close()

        # ---------- P3: expert FFN ----------
        with tc.tile_pool(name="ffn", bufs=2) as ffn, \
             tc.tile_pool(name="htp", bufs=1) as htp, \
             tc.tile_pool(name="ps1", bufs=2, space="PSUM") as ps1p, \
             tc.tile_pool(name="ps2", bufs=2, space="PSUM") as ps2p, \
             tc.tile_pool(name="pst", bufs=2, space="PSUM") as pstp:
            for c in range(NCHUNK):
                xg = ffn.tile([128, NBLK, D], BF16, tag="xg")
                nc.gpsimd.dma_gather(
                    out_ap=xg,
                    in_ap=x_bf16[:, :],
                    idxs_ap=dIdx[:, c * (CB // 16):(c + 1) * (CB // 16)],
                    num_idxs=CB,
                    num_idxs_reg=CB,
                    elem_size=D,
                    transpose=False)
                xTb = ffn.tile([128, DCH, CB], BF16, tag="xTb")
                for dch in range(DCH):
                    psT = pstp.tile([128, CB], BF16, tag="psT")
                    for blk in range(NBLK):
                        nc.tensor.transpose(
                            psT[:, blk * 128:(blk + 1) * 128],
                            xg[:, blk, dch * 128:(dch + 1) * 128],
                            ident_b)
                    nc.vector.tensor_copy(xTb[:, dch, :], psT)
                hT = htp.tile([128, HCH, CB], BF16, tag="hT")
                for j in range(HCH):
                    ps1 = ps1p.tile([128, CB], FP32, tag="ps1")
                    for dch in range(DCH):
                        nc.tensor.matmul(
                            ps1, lhsT=W1s[:, dch, j * 128:(j + 1) * 128],
                            rhs=xTb[:, dch, :],
                            start=(dch == 0), stop=(dch == DCH - 1))
                    nc.scalar.activation(hT[:, j, :], ps1, AF.Silu,
                                         bias=b1s[:, j:j + 1])
                for cs in range(NBLK):
                    osb = ffn.tile([128, D], BF16, tag="osb")
                    for dh in range(2):
                        ps2 = ps2p.tile([128, 512], FP32, tag="ps2")
                        for j in range(HCH):
                            nc.tensor.matmul(
                                ps2,
                                lhsT=hT[:, j, cs * 128:(cs + 1) * 128],
                                rhs=W2s[:, j, dh * 512:(dh + 1) * 512],
                                start=(j == 0), stop=False)
                        nc.tensor.matmul(
                            ps2, lhsT=ones1b,
                            rhs=b2s[:, dh * 512:(dh + 1) * 512],
                            start=False, stop=True)
                        blk = c * NBLK + cs
                        nc.vector.tensor_scalar(
                            osb[:, dh * 512:(dh + 1) * 512], ps2,
                            gdp[:, blk:blk + 1], None, op0=ALU.mult)
                    nc.sync.dma_start(
                        out_e[(c * NBLK + cs) * 128:
                              (c * NBLK + cs + 1) * 128, :],
                        osb)
                if AG_CHUNKED:
                    nc.gpsimd.collective_compute(
                        "AllGather", ALU.bypass,
                        replica_groups=[list(range(NC))],
                        ins=[out_e[c * CB:(c + 1) * CB, :]],
                        outs=[all_out[c]])
            if not AG_CHUNKED:
                nc.gpsimd.collective_compute(
                    "AllGather", ALU.bypass,
                    replica_groups=[list(range(NC))],
                    ins=[out_e.opt()], outs=[all_out.opt()])
            if dbg:
                nc.sync.dma_start(dbg_oe[:, :], out_e)

        # ---------- P4: combine own shard ----------
        with tc.tile_pool(name="comb", bufs=2) as comb:
            if AG_CHUNKED:
                allv = all_out.rearrange("n r c d -> (n r c) d")
            else:
                allv = all_out.rearrange("r c d -> (r c) d")
            GC = 4  # t-tiles per gather (512 idxs)
            for t0 in range(0, TT, GC):
                gk = []
                for k in range(K):
                    gkt = comb.tile([128, GC, D], BF16, tag=f"gk{k}")
                    gk.append(gkt)
                    nc.gpsimd.dma_gather(
                        out_ap=gkt,
                        in_ap=allv,
                        idxs_ap=ci16[:, (k * TT + t0) * 8:
                                     (k * TT + t0 + GC) * 8],
                        num_idxs=GC * 128,
                        num_idxs_reg=GC * 128,
                        elem_size=D,
                        transpose=False)
                for tr in range(GC):
                    t = t0 + tr
                    ysb = comb.tile([128, D], FP32, tag="ysb")
                    nc.vector.tensor_tensor(ysb, gk[0][:, tr, :],
                                            gk[1][:, tr, :], ALU.add)
                    nc.sync.dma_start(y_out[t * 128:(t + 1) * 128, :], ysb)

        top.close()

    nc.compile()
    return nc


# ---------------- host-side staging ----------------

def bfloat16_np():
    import ml_dtypes
    return ml_dtypes.bfloat16


def stage_inputs(cfg: Cfg, x, Wr, br, W1, b1, W2, b2):
    """x: [N, D] fp32; returns list of per-core input dicts."""
    E, D, H, TPC, NC = cfg.E, cfg.D, cfg.H, cfg.TPC, cfg.ncores
    DCH, HCH, TT = cfg.DCH, cfg.HCH, cfg.TT
    x = np.ascontiguousarray(x, np.float32)
    x_bf = x.astype(bfloat16_np())
    ltri = np.tril(np.ones((128, 128), np.float32), -1).astype(bfloat16_np())
    in_maps = []
    for r in range(NC):
        shard = x[r * TPC:(r + 1) * TPC]
        m = {
            "xT_shard": np.ascontiguousarray(shard.T),
            "x_bf16": x_bf,
            "Wr_in": np.ascontiguousarray(
                Wr.reshape(DCH, 128, E).transpose(1, 0, 2)).astype(np.float32),
            "br_in": br.reshape(1, E).astype(np.float32),
            "W1_in": np.ascontiguousarray(
                W1[r].reshape(DCH, 128, H).transpose(1, 0, 2)
            ).astype(bfloat16_np()),
            "W2_in": np.ascontiguousarray(
                W2[r].reshape(HCH, 128, D).transpose(1, 0, 2)
            ).astype(bfloat16_np()),
            "b1_in": np.ascontiguousarray(
                b1[r].reshape(HCH, 128).T).astype(np.float32),
            "b2_in": b2[r].reshape(1, D).astype(np.float32).astype(
                bfloat16_np()),
            "ltri_in": ltri,
            "rk_in": np.full((128, 1), r * TT, np.float32),
        }
        in_maps.append(m)
    return in_maps


# ---------------- problem binding ----------------

import math as _math

B, T = 8, 2048
_N = B * T
_D = 1024
_CAP = int(_math.ceil(1.2 * _N / 8))  # 2458

_CACHE = {}


def _get_nc():
    if "nc" not in _CACHE:
        cfg = Cfg(D=_D, H=4096, TPC=_N // 8, cap=_CAP, CAPP=2560, CB=512)
        _CACHE["cfg"] = cfg
        _CACHE["nc"] = build(cfg, dbg=bool(int(os.environ.get("KDBG", "0"))))
    return _CACHE["cfg"], _CACHE["nc"]


TRACE = False
_LAST_EXEC_NS = None
_LAST_RES = None


def kernel(x_btd, Wr, br, W1, b1, W2, b2):
    from concourse.bass_utils import run_bass_kernel_spmd

    global _LAST_EXEC_NS, _LAST_RES
    cfg, nc = _get_nc()
    x = np.ascontiguousarray(np.asarray(x_btd), np.float32).reshape(_N, _D)
    in_maps = stage_inputs(
        cfg, x, np.asarray(Wr), np.asarray(br), np.asarray(W1),
        np.asarray(b1), np.asarray(W2), np.asarray(b2))
    if TRACE:
        import shutil
        tdir = "/root/problem/work/trace"
        shutil.rmtree(tdir, ignore_errors=True)
        os.makedirs(tdir, exist_ok=True)
        tcores = (list(range(8))
                  if os.environ.get("KTRACE_ALL", "0") == "1" else [0])
        res = run_bass_kernel_spmd(nc, in_maps, list(range(8)), trace=True,
                                   trace_cores=tcores, tmpdir=tdir)
        _LAST_RES = res
        if getattr(res, "exec_time_ns", None):
            _LAST_EXEC_NS = res.exec_time_ns
    else:
        res = run_bass_kernel_spmd(nc, in_maps, list(range(8)))
        _LAST_RES = res
    ys = [res.results[r]["y_out"] for r in range(8)]
    y = np.concatenate(ys, axis=0).astype(np.float32)
    return y.reshape(B, T, _D)


# revision 66
# speedup vs baseline: 1.1932x; 1.0067x over previous
"""TRN2 Bass kernel for nn_MoEPositionwiseFFN: kernel(**inputs) -> np.ndarray.

v2: expert-parallel MoE FFN without dynamic-DMA dispatch scatter.

Per core r (= expert r):
  P1  router on own 2048 tokens (fp32), AllGather 4 routing planes.
  P2  replicated capacity selection (threshold bisection) -> keep/pos.
  P2.5 own-token extraction; slot->token dispatch table built with
       one-hot rank-1 matmuls into PSUM; ReduceScatter(add) routes each
       expert its [CAPP] slice. Combine indices built via PE transposes.
  P3  expert FFN on 2560 gathered rows (row dma_gather + PE transpose),
       output AllGather chunked (5x) to overlap with compute.
  P4  combine: gather 2 rows/token from all_out, gate-weighted sum
       (gates applied combine-side; ZSLOT row zeroed explicitly).
"""

import os
import sys

for _p in ("/opt/trn_rl_repo", "/opt/pypackages"):
    if _p not in sys.path:
        sys.path.insert(0, _p)


from dataclasses import dataclass

import numpy as np

import concourse.bass as bass
import concourse.bacc as bacc
import concourse.tile as tile
import concourse.mybir as mybir
from concourse.masks import make_identity

FP32 = mybir.dt.float32
BF16 = mybir.dt.bfloat16
I32 = mybir.dt.int32
I16 = mybir.dt.int16
U16 = mybir.dt.uint16
AF = mybir.ActivationFunctionType
ALU = mybir.AluOpType
AX = mybir.AxisListType


@dataclass
class Cfg:
    ncores: int = 8
    E: int = 8
    K: int = 2
    D: int = 1024
    H: int = 4096
    TPC: int = 2048          # tokens per core
    cap: int = 2458          # reference capacity
    CAPP: int = 2560         # padded capacity (= NCHUNK*CB)
    CB: int = 512            # FFN chunk / AllGather chunk (tokens)
    NBIS: int = 26           # bisection iterations (covers 0x3C000000..0x3F800000)

    @property
    def N(self):
        return self.ncores * self.TPC

    @property
    def TT(self):
        return self.TPC // 128  # token tiles per core (16)

    @property
    def M(self):
        return self.N // 128    # global token groups (128)

    @property
    def DCH(self):
        return self.D // 128

    @property
    def HCH(self):
        return self.H // 128

    @property
    def NCHUNK(self):
        return self.CAPP // self.CB

    @property
    def FW(self):
        return self.CAPP // 16  # dispatch-table free width (160)


def build(cfg: Cfg, dbg: bool = False):
    E, K, D, H = cfg.E, cfg.K, cfg.D, cfg.H
    TPC, TT, M, N = cfg.TPC, cfg.TT, cfg.M, cfg.N
    DCH, HCH = cfg.DCH, cfg.HCH
    CAP, CAPP, CB, NCHUNK, FW = cfg.cap, cfg.CAPP, cfg.CB, cfg.NCHUNK, cfg.FW
    NC = cfg.ncores
    assert E == NC == 8 and K == 2
    assert CAPP == NCHUNK * CB and CAPP % 16 == 0 and CB % 128 == 0
    NBLK = CB // 128          # 128-token blocks per chunk (4)
    # combine flat row for dropped assignments: chunk layout row of
    # (expert 0, slot CAPP-1), guaranteed zeroed.
    ZC = (CAPP - 1) // CB
    ZOFF = ZC * (NC * CB) + 0 * CB + ((CAPP - 1) - ZC * CB)

    nc = bacc.Bacc("TRN2", target_bir_lowering=False, debug=False,
                   num_devices=NC)

    # ---- external inputs (per-core staged by host) ----
    xT_shard = nc.dram_tensor("xT_shard", [D, TPC], FP32, kind="ExternalInput")
    x_bf16 = nc.dram_tensor("x_bf16", [N, D], BF16, kind="ExternalInput")
    Wr_in = nc.dram_tensor("Wr_in", [128, DCH, E], FP32, kind="ExternalInput")
    br_in = nc.dram_tensor("br_in", [1, E], FP32, kind="ExternalInput")
    W1_in = nc.dram_tensor("W1_in", [128, DCH, H], BF16, kind="ExternalInput")
    W2_in = nc.dram_tensor("W2_in", [128, HCH, D], BF16, kind="ExternalInput")
    b1_in = nc.dram_tensor("b1_in", [128, HCH], FP32, kind="ExternalInput")
    b2_in = nc.dram_tensor("b2_in", [1, D], BF16, kind="ExternalInput")
    ltri_in = nc.dram_tensor("ltri_in", [128, 128], BF16, kind="ExternalInput")
    rk_in = nc.dram_tensor("rk_in", [128, 1], FP32, kind="ExternalInput")

    # ---- external output ----
    y_out = nc.dram_tensor("y_out", [TPC, D], FP32, kind="ExternalOutput")

    if dbg:
        dbg_rta = nc.dram_tensor("dbg_rta", [4, 128, M], FP32,
                                 kind="ExternalOutput")
        dbg_pos = nc.dram_tensor("dbg_pos", [128, E * M], FP32,
                                 kind="ExternalOutput")
        dbg_keep = nc.dram_tensor("dbg_keep", [128, E * M], FP32,
                                  kind="ExternalOutput")
        dbg_tab = nc.dram_tensor("dbg_tab", [16, 2 * FW], FP32,
                                 kind="ExternalOutput")
        dbg_cidx = nc.dram_tensor("dbg_cidx", [128, K * TT * 8], I16,
                                  kind="ExternalOutput")
        dbg_didx = nc.dram_tensor("dbg_didx", [128, FW], I16,
                                  kind="ExternalOutput")
        dbg_oe = nc.dram_tensor("dbg_oe", [CAPP, D], BF16,
                                kind="ExternalOutput")

    with tile.TileContext(nc) as tc:
        rank_sp = nc.partition_id()

        import contextlib
        top = contextlib.ExitStack()
        cpool = top.enter_context(tc.tile_pool(name="const", bufs=1))
        wts = top.enter_context(tc.tile_pool(name="wts", bufs=1))
        keepp = top.enter_context(tc.tile_pool(name="keepp", bufs=1))
        dramp = top.enter_context(tc.tile_pool(name="dramp", bufs=1,
                                               space="DRAM"))

        # ---- DRAM scratch ----
        rt_local = dramp.tile([4, 128, TT], FP32, tag="rt_local")
        rt_all = dramp.tile([NC, 4, 128, TT], FP32, tag="rt_all",
                            addr_space="Shared")
        tab_dram = dramp.tile([128, 2 * FW], FP32, tag="tab_dram")
        tab_all = dramp.tile([NC, 128, 2 * FW], FP32, tag="tab_all",
                             addr_space="Shared")
        out_e = dramp.tile([CAPP, D], BF16, tag="out_e")
        AG_CHUNKED = True
        if AG_CHUNKED:
            all_out = dramp.tile([NCHUNK, NC, CB, D], BF16, tag="all_out")
        else:
            all_out = dramp.tile([NC, CAPP, D], BF16, tag="all_out",
                                 addr_space="Shared")

        # ---- weights (DMA issued after router loads; overlaps P2) ----
        W1s = wts.tile([128, DCH, H], BF16, tag="W1s")
        W2s = wts.tile([128, HCH, D], BF16, tag="W2s")
        b1s = wts.tile([128, HCH], FP32, tag="b1s")
        b2s = wts.tile([1, D], BF16, tag="b2s")

        # ---- constants ----
        ident_b = cpool.tile([128, 128], BF16, tag="ident_b")
        make_identity(nc, ident_b)
        ident_f = cpool.tile([128, 128], FP32, tag="ident_f")
        make_identity(nc, ident_f)
        ltri = cpool.tile([128, 128], BF16, tag="ltri")
        nc.sync.dma_start(ltri, ltri_in[:, :])
        wr_sb = cpool.tile([128, DCH, E], FP32, tag="wr")
        nc.sync.dma_start(wr_sb, Wr_in[:, :, :])
        br_sb = cpool.tile([1, E], FP32, tag="br")
        nc.sync.dma_start(br_sb, br_in[:, :])
        rk_sb = cpool.tile([128, 1], FP32, tag="rk_sb")
        nc.sync.dma_start(rk_sb, rk_in[:, :])
        ones1f = cpool.tile([1, 128], FP32, tag="ones1f")
        nc.vector.memset(ones1f, 1.0)
        ones1b = cpool.tile([1, 128], BF16, tag="ones1b")
        nc.vector.memset(ones1b, 1.0)
        ones128b = cpool.tile([128, 128], BF16, tag="ones128b")
        nc.vector.memset(ones128b, 1.0)
        one_i = cpool.tile([128, E], I32, tag="one_i")
        nc.vector.memset(one_i, 1)
        # iota along free: F160[p, j] = j ; F128 = F160[:, :128]
        it_i = cpool.tile([128, FW], I32, tag="it_i")
        nc.gpsimd.iota(it_i, pattern=[[1, FW]], base=0, channel_multiplier=0)
        F160 = cpool.tile([128, FW], FP32, tag="F160")
        nc.vector.tensor_copy(F160, it_i)
        F128 = F160[:, 0:128]
        # lovals[p, 0] = p
        lov_i = cpool.tile([128, 1], I32, tag="lov_i")
        nc.gpsimd.iota(lov_i, pattern=[[1, 1]], base=0, channel_multiplier=1)
        lovals = cpool.tile([128, 1], FP32, tag="lovals")
        nc.vector.tensor_copy(lovals, lov_i)
        # hival[p, t] = rank*TT + t   (token-group id of own tile t)
        tt_i = cpool.tile([128, TT], I32, tag="tt_i")
        nc.gpsimd.iota(tt_i, pattern=[[1, TT]], base=0, channel_multiplier=0)
        hival = cpool.tile([128, TT], FP32, tag="hival")
        nc.vector.tensor_copy(hival, tt_i)
        nc.vector.tensor_scalar(hival, hival, rk_sb, None, op0=ALU.add)
        # zmask[p] = 0 for p == 127 else 1 (ZSLOT row kill)
        zmask = cpool.tile([128, 1], FP32, tag="zmask")
        nc.vector.tensor_scalar(zmask, lovals, 127.0, None,
                                op0=ALU.not_equal)
        # tokv[p, t] = global token id of own (t, p) = (rk+t)*128 + p
        tokv = cpool.tile([128, TT], FP32, tag="tokv")
        nc.vector.tensor_scalar(tokv, hival, 128.0, lovals,
                                op0=ALU.mult, op1=ALU.add)

        # ---- persistent small tiles (survive into P3/P4) ----
        dIdx = keepp.tile([128, FW], I16, tag="dIdx")
        ci16 = keepp.tile([128, K * TT * 8], I16, tag="ci16")
        g1o = keepp.tile([128, TT], FP32, tag="g1o")
        g2o = keepp.tile([128, TT], FP32, tag="g2o")
        gdp = keepp.tile([128, NCHUNK * NBLK], FP32, tag="gdp")

        selstack = contextlib.ExitStack()
        sel = selstack.enter_context(tc.tile_pool(name="sel", bufs=1))
        lrp = selstack.enter_context(tc.tile_pool(name="lrp", bufs=2))
        psr = selstack.enter_context(
            tc.tile_pool(name="psr", bufs=2, space="PSUM"))
        pscnt = selstack.enter_context(
            tc.tile_pool(name="pscnt", bufs=2, space="PSUM"))
        pstab = selstack.enter_context(
            tc.tile_pool(name="pstab", bufs=1, space="PSUM"))
        pstr = selstack.enter_context(
            tc.tile_pool(name="pstr", bufs=1, space="PSUM"))
        xstack = contextlib.ExitStack()
        xpool = xstack.enter_context(tc.tile_pool(name="xpool", bufs=3))

        # ---------- P1: router on own shard ----------
        # logits via Wr-stationary matmuls: ps8[e, tok] = sum_d Wr[d,e]x[d,tok]
        br8 = cpool.tile([8, 1], FP32, tag="br8")
        nc.sync.dma_start(br8, br_in.rearrange("o e -> e o"))
        E_sb = sel.tile([128, TT, E], FP32, tag="E_sb")
        QT = TT // 4  # t-tiles per quarter (512 tokens)
        for q4 in range(4):
            xq = xpool.tile([128, DCH, QT * 128], FP32, tag="xq")
            nc.sync.dma_start(
                xq,
                xT_shard[:, q4 * QT * 128:(q4 + 1) * QT * 128]
                .rearrange("(dch p) t -> p dch t", p=128))
            ps8 = psr.tile([8, QT * 128], FP32, tag="ps8")
            for dch in range(DCH):
                nc.tensor.matmul(ps8, lhsT=wr_sb[:, dch, :],
                                 rhs=xq[:, dch, :],
                                 start=(dch == 0), stop=(dch == DCH - 1))
            sb8 = xpool.tile([8, QT * 128], FP32, tag="sb8")
            nc.scalar.activation(sb8, ps8, AF.Identity, bias=br8)
            for tr in range(QT):
                t = q4 * QT + tr
                psT = pstr.tile([128, 8], FP32, tag="psT8")
                nc.tensor.transpose(psT, sb8[:, tr * 128:(tr + 1) * 128],
                                    ident_f[0:8, 0:8])
                nc.scalar.activation(E_sb[:, t, :], psT, AF.Exp)
        xstack.close()
        selB = selstack.enter_context(tc.tile_pool(name="selB", bufs=1))
        # batched top-2 over the expert axis
        e8i = sel.tile([128, TT * 8], I16, tag="e8i")
        nc.gpsimd.iota(e8i, pattern=[[0, TT], [1, 8]], base=0,
                       channel_multiplier=0)
        e8f = sel.tile([128, TT, 8], FP32, tag="e8f")
        nc.vector.tensor_copy(e8f.rearrange("p a b -> p (a b)"), e8i)
        Z_sb = sel.tile([128, TT], FP32, tag="Z_sb")
        nc.vector.tensor_reduce(Z_sb, E_sb, AX.X, ALU.add)
        m1 = sel.tile([128, TT], FP32, tag="m1")
        nc.vector.tensor_reduce(m1, E_sb, AX.X, ALU.max)
        eqx = sel.tile([128, TT, 8], FP32, tag="eqx")
        nc.vector.tensor_tensor(
            eqx, E_sb, m1.unsqueeze(2).broadcast_to((128, TT, 8)),
            ALU.is_equal)
        tmp8 = sel.tile([128, TT, 8], FP32, tag="tmp8")
        nc.vector.tensor_tensor(tmp8, eqx, e8f, ALU.mult)
        P_i1 = sel.tile([128, TT], FP32, tag="P_i1")
        nc.vector.tensor_reduce(P_i1, tmp8, AX.X, ALU.add)
        nc.vector.tensor_scalar(tmp8, eqx, -1e30, None, op0=ALU.mult)
        nc.vector.tensor_tensor(E_sb, E_sb, tmp8, ALU.add)  # mask out top-1
        m2 = sel.tile([128, TT], FP32, tag="m2")
        nc.vector.tensor_reduce(m2, E_sb, AX.X, ALU.max)
        nc.vector.tensor_tensor(
            eqx, E_sb, m2.unsqueeze(2).broadcast_to((128, TT, 8)),
            ALU.is_equal)
        nc.vector.tensor_tensor(tmp8, eqx, e8f, ALU.mult)
        P_i2 = sel.tile([128, TT], FP32, tag="P_i2")
        nc.vector.tensor_reduce(P_i2, tmp8, AX.X, ALU.add)
        rZ = sel.tile([128, TT], FP32, tag="rZ")
        nc.vector.reciprocal(rZ, Z_sb)
        P_g1 = sel.tile([128, TT], FP32, tag="P_g1")
        P_g2 = sel.tile([128, TT], FP32, tag="P_g2")
        nc.vector.tensor_tensor(P_g1, m1, rZ, ALU.mult)
        nc.vector.tensor_tensor(P_g2, m2, rZ, ALU.mult)
        nc.sync.dma_start(rt_local[0], P_i1)
        nc.sync.dma_start(rt_local[1], P_g1)
        nc.sync.dma_start(rt_local[2], P_i2)
        nc.sync.dma_start(rt_local[3], P_g2)
        nc.gpsimd.collective_compute(
            "AllGather", ALU.bypass,
            replica_groups=[list(range(NC))],
            ins=[rt_local.opt()], outs=[rt_all.opt()])
        nc.sync.dma_start(W1s, W1_in[:, :, :])
        nc.sync.dma_start(W2s, W2_in[:, :, :])
        nc.sync.dma_start(b1s, b1_in[:, :])
        nc.sync.dma_start(b2s, b2_in[:, :])

        # ---------- P2: replicated selection ----------
        i1f = sel.tile([128, M], FP32, tag="i1f")
        g1f = sel.tile([128, M], FP32, tag="g1f")
        i2f = sel.tile([128, M], FP32, tag="i2f")
        g2f = sel.tile([128, M], FP32, tag="g2f")
        for q, dst in ((0, i1f), (1, g1f), (2, i2f), (3, g2f)):
            nc.sync.dma_start(
                dst.rearrange("p (r t) -> p r t", r=NC),
                rt_all[:, q, :, :].rearrange("r p t -> p r t"))
        if dbg:
            nc.sync.dma_start(dbg_rta[0], i1f)
            nc.sync.dma_start(dbg_rta[1], g1f)
            nc.sync.dma_start(dbg_rta[2], i2f)
            nc.sync.dma_start(dbg_rta[3], g2f)

        A_sb = selB.tile([128, E, M], FP32, tag="A_sb")
        tmpM = sel.tile([128, M], FP32, tag="tmpM")
        for e in range(E):
            nc.vector.scalar_tensor_tensor(
                A_sb[:, e, :], i1f, float(e), g1f,
                op0=ALU.is_equal, op1=ALU.mult)
            nc.vector.scalar_tensor_tensor(
                tmpM, i2f, float(e), g2f, op0=ALU.is_equal, op1=ALU.mult)
            nc.vector.tensor_tensor(A_sb[:, e, :], A_sb[:, e, :], tmpM,
                                    ALU.add)

        big = selB.tile([128, E, M], FP32, tag="big")
        cntp = sel.tile([128, E], FP32, tag="cntp")
        cntb = sel.tile([128, E], BF16, tag="cntb")
        cntf = sel.tile([128, E], FP32, tag="cntf")
        Ktgt = sel.tile([128, E], FP32, tag="Ktgt")
        lo = sel.tile([128, E], I32, tag="lo")
        hi = sel.tile([128, E], I32, tag="hi")
        mid = sel.tile([128, E], I32, tag="mid")
        condi = sel.tile([128, E], I32, tag="condi")

        nc.vector.tensor_scalar(big, A_sb, 0.0, None, op0=ALU.is_gt)
        nc.vector.tensor_reduce(cntp, big, AX.X, ALU.add)
        nc.vector.tensor_copy(cntb, cntp)
        pc = pscnt.tile([128, E], FP32, tag="pscnt")
        nc.tensor.matmul(pc, lhsT=ones128b, rhs=cntb, start=True, stop=True)
        nc.vector.tensor_scalar(Ktgt, pc, float(CAP), None, op0=ALU.min)

        zerosM = selB.tile([128, M], FP32, tag="zerosM")
        nc.vector.memset(zerosM, 0.0)
        nc.vector.memset(lo, 0x3C000000)
        nc.vector.memset(hi, 0x3F800000)
        for it in range(cfg.NBIS):
            nc.vector.tensor_tensor(mid, lo, hi, ALU.add)
            nc.vector.tensor_tensor(mid, mid, one_i,
                                    ALU.logical_shift_right)
            midf = mid.bitcast(FP32)
            for e in range(E):
                nc.vector.scalar_tensor_tensor(
                    big[:, e, :], A_sb[:, e, :], midf[:, e:e + 1], zerosM,
                    op0=ALU.is_gt, op1=ALU.add,
                    accum_out=cntp[:, e:e + 1])
            nc.vector.tensor_copy(cntb, cntp)
            pc = pscnt.tile([128, E], FP32, tag="pscnt")
            nc.tensor.matmul(pc, lhsT=ones128b, rhs=cntb, start=True,
                             stop=True)
            nc.vector.tensor_copy(cntf, pc)
            nc.vector.tensor_tensor(condi, cntf, Ktgt, ALU.is_ge)
            nc.vector.copy_predicated(lo, condi, mid)
            nc.vector.tensor_tensor(condi, cntf, Ktgt, ALU.is_lt)
            nc.vector.copy_predicated(hi, condi, mid)

        thrf = lo.bitcast(FP32)
        keepf = selB.tile([128, E, M], FP32, tag="keepf")
        nc.vector.tensor_tensor(
            keepf, A_sb, thrf.unsqueeze(2).broadcast_to((128, E, M)),
            ALU.is_gt)

        rp = selB.tile([128, E, M], FP32, tag="rp")
        for e in range(E):
            nc.vector.tensor_tensor_scan(
                rp[:, e, :], keepf[:, e, :], zerosM, initial=0.0,
                op0=ALU.add, op1=ALU.add)
        totb = sel.tile([128, E], BF16, tag="totb")
        nc.vector.tensor_copy(totb, rp[:, :, M - 1])
        pe_x = pscnt.tile([128, E], FP32, tag="pscnt")
        nc.tensor.matmul(pe_x, lhsT=ltri, rhs=totb, start=True, stop=True)
        excl = sel.tile([128, E], FP32, tag="excl")
        nc.vector.tensor_copy(excl, pe_x)
        pos = selB.tile([128, E, M], FP32, tag="pos")
        nc.vector.tensor_tensor(pos, rp, keepf, ALU.subtract)
        nc.vector.tensor_tensor(
            pos, pos, excl.unsqueeze(2).broadcast_to((128, E, M)),
            ALU.add)
        if dbg:
            nc.sync.dma_start(dbg_pos[:, :],
                              pos.rearrange("p e m -> p (e m)"))
            nc.sync.dma_start(dbg_keep[:, :],
                              keepf.rearrange("p e m -> p (e m)"))

        # ---------- P2.5: own-token extraction + dispatch table ----------
        own0 = bass.ds(rank_sp * TT, TT)
        c15 = sel.tile([128, TT], I32, tag="c15")
        nc.vector.memset(c15, 15)
        c4 = sel.tile([128, TT], I32, tag="c4")
        nc.vector.memset(c4, 4)
        c511 = sel.tile([128, TT], I32, tag="c511")
        nc.vector.memset(c511, 511)
        c9 = sel.tile([128, TT], I32, tag="c9")
        nc.vector.memset(c9, 9)
        pl_k = []      # [128, TT] fp32 per k: table row  e*16 + pos%16
        fs_k = []      # [128, TT] fp32 per k: table col  pos//16 (999=dead)
        offall = sel.tile([128, K * TT], FP32, tag="offall")
        tmpT = sel.tile([128, TT], FP32, tag="tmpT")
        for k in range(K):
            ikf = i1f if k == 0 else i2f
            gkf = g1f if k == 0 else g2f
            go = g1o if k == 0 else g2o
            nc.vector.tensor_copy(go, gkf[:, own0])
            iko = sel.tile([128, TT], FP32, tag=f"iko{k}")
            nc.vector.tensor_copy(iko, ikf[:, own0])
            posk = sel.tile([128, TT], FP32, tag=f"posk{k}")
            keepk = sel.tile([128, TT], FP32, tag=f"keepk{k}")
            first = True
            for e in range(E):
                dst = posk if first else tmpT
                nc.vector.scalar_tensor_tensor(
                    dst, iko, float(e), pos[:, e, own0],
                    op0=ALU.is_equal, op1=ALU.mult)
                if not first:
                    nc.vector.tensor_tensor(posk, posk, tmpT, ALU.add)
                first = False
            first = True
            for e in range(E):
                dst = keepk if first else tmpT
                nc.vector.scalar_tensor_tensor(
                    dst, iko, float(e), keepf[:, e, own0],
                    op0=ALU.is_equal, op1=ALU.mult)
                if not first:
                    nc.vector.tensor_tensor(keepk, keepk, tmpT, ALU.add)
                first = False
            keepi = sel.tile([128, TT], I32, tag=f"keepi{k}")
            nc.vector.tensor_copy(keepi, keepk)
            # table coords (integer split of pos: %16 and //16)
            posI = sel.tile([128, TT], I32, tag=f"posI{k}")
            nc.vector.tensor_copy(posI, posk)
            tmpI = sel.tile([128, TT], I32, tag=f"tmpI{k}")
            nc.vector.tensor_tensor(tmpI, posI, c15, ALU.bitwise_and)
            qo = sel.tile([128, TT], FP32, tag=f"qo{k}")
            nc.vector.tensor_copy(qo, tmpI)
            nc.vector.tensor_tensor(tmpI, posI, c4, ALU.logical_shift_right)
            fo = sel.tile([128, TT], FP32, tag=f"fo{k}")
            nc.vector.tensor_copy(fo, tmpI)
            plo = sel.tile([128, TT], FP32, tag=f"plo{k}")
            nc.vector.scalar_tensor_tensor(
                plo, iko, 16.0, qo, op0=ALU.mult, op1=ALU.add)
            fsel = sel.tile([128, TT], FP32, tag=f"fsel{k}")
            nc.vector.memset(fsel, 999.0)
            nc.vector.copy_predicated(fsel, keepi, fo)
            pl_k.append(plo)
            fs_k.append(fsel)
            off = sel.tile([128, TT], FP32, tag=f"off{k}")
            if AG_CHUNKED:
                # combine flat row (chunk layout): c = pos//CB;
                # off = c*(NC*CB) + ik*CB + pos%CB ; dropped -> ZOFF
                nc.vector.tensor_tensor(tmpI, posI, c511, ALU.bitwise_and)
                m5 = sel.tile([128, TT], FP32, tag=f"m5{k}")
                nc.vector.tensor_copy(m5, tmpI)
                nc.vector.tensor_tensor(tmpI, posI, c9,
                                        ALU.logical_shift_right)
                cdv = sel.tile([128, TT], FP32, tag=f"cdv{k}")
                nc.vector.tensor_copy(cdv, tmpI)
                nc.vector.tensor_scalar(cdv, cdv, float(NC * CB), None,
                                        op0=ALU.mult)
                nc.vector.scalar_tensor_tensor(
                    off, iko, float(CB), m5, op0=ALU.mult, op1=ALU.add)
                nc.vector.tensor_tensor(off, off, cdv, ALU.add)
                zoff = float(ZOFF)
            else:
                # off = ik*CAPP + pos ; dropped -> expert0 slot CAPP-1
                nc.vector.scalar_tensor_tensor(
                    off, iko, float(CAPP), posk, op0=ALU.mult, op1=ALU.add)
                zoff = float(CAPP - 1)
            offd = sel.tile([128, TT], FP32, tag=f"offd{k}")
            nc.vector.memset(offd, zoff)
            nc.vector.copy_predicated(offd, keepi, off)
            nc.vector.tensor_copy(offall[:, k * TT:(k + 1) * TT], offd)

        # table build: psTab[row, :] += sum over items of onehot outer
        # (fp32 planes: [0:FW] token id, [FW:2FW] gate)
        psTab = pstab.tile([128, 2 * FW], FP32, tag="psTab")
        nck = 0
        for k in range(K):
            go = g1o if k == 0 else g2o
            for t in range(TT):
                L = lrp.tile([128, 128], FP32, tag="L")
                nc.vector.tensor_scalar(
                    L, F128, pl_k[k][:, t:t + 1], None, op0=ALU.is_equal)
                R = lrp.tile([128, 2 * FW], FP32, tag="R")
                nc.vector.tensor_scalar(
                    R[:, 0:FW], F160, fs_k[k][:, t:t + 1], tokv[:, t:t + 1],
                    op0=ALU.is_equal, op1=ALU.mult)
                nc.vector.tensor_scalar(
                    R[:, FW:2 * FW], F160, fs_k[k][:, t:t + 1],
                    go[:, t:t + 1], op0=ALU.is_equal, op1=ALU.mult)
                nc.tensor.matmul(psTab, lhsT=L, rhs=R,
                                 start=(nck == 0), stop=(nck == K * TT - 1))
                nck += 1
        tabsb = selB.tile([128, 2 * FW], FP32, tag="tabsb")
        nc.vector.tensor_copy(tabsb, psTab)
        nc.sync.dma_start(tab_dram, tabsb)
        nc.gpsimd.collective_compute(
            "AllGather", ALU.bypass,
            replica_groups=[list(range(NC))],
            ins=[tab_dram.opt()], outs=[tab_all.opt()])

        # readback own expert's 16 rows from each core's table and sum
        own16 = bass.ds(rank_sp * 16, 16)
        tabs = selB.tile([16, 2 * FW], FP32, tag="tabs")
        tabr = selB.tile([16, NC, 2 * FW], FP32, tag="tabr")
        for r in range(NC):
            nc.sync.dma_start(tabr[:, r, :], tab_all[r, own16, :])
        nc.vector.tensor_tensor(tabs, tabr[:, 0, :], tabr[:, 1, :], ALU.add)
        for r in range(2, NC):
            nc.vector.tensor_tensor(tabs, tabs, tabr[:, r, :], ALU.add)
        if dbg:
            nc.sync.dma_start(dbg_tab, tabs)
        dI16 = sel.tile([16, FW], I16, tag="dI16")
        nc.vector.tensor_copy(dI16, tabs[:, 0:FW])
        # per-slot gates, relayout [16q, 160f] -> [128 = (f%8)*16+q, f//8]
        gview = tabs[:, FW:2 * FW].rearrange("q (fd fm) -> q fd fm", fm=8)
        for fm in range(8):
            nc.sync.dma_start(gdp[fm * 16:(fm + 1) * 16, :],
                              gview[:, :, fm])
        for g in range(8):
            nc.sync.dma_start(dIdx[g * 16:(g + 1) * 16, :], dI16)

        # combine idx relayout via PE transposes:
        # cidxf[pl, (k t), ph] = offall[ph*16+pl, (k t)]
        psO = pstr.tile([32, 128], FP32, tag="psO")
        nc.tensor.transpose(psO, offall, ident_f)
        T1 = sel.tile([32, 128], FP32, tag="T1")
        nc.vector.tensor_copy(T1, psO)
        cidxf = sel.tile([128, K * TT, 8], FP32, tag="cidxf")
        for ph in range(8):
            psP = pstr.tile([16, 32], FP32, tag="psP")
            nc.tensor.transpose(psP, T1[:, ph * 16:(ph + 1) * 16],
                                ident_f[0:32, 0:32])
            nc.vector.tensor_copy(cidxf[0:16, :, ph], psP)
        cs16 = sel.tile([16, K * TT * 8], I16, tag="cs16")
        nc.vector.tensor_copy(
            cs16, cidxf[0:16].rearrange("p a b -> p (a b)"))
        for g in range(8):
            nc.sync.dma_start(ci16[g * 16:(g + 1) * 16, :], cs16)
        if dbg:
            nc.sync.dma_start(dbg_cidx, ci16)
            nc.sync.dma_start(dbg_didx, dIdx)

        selstack.close()

        # ---------- P3: expert FFN ----------
        with tc.tile_pool(name="ffn", bufs=2) as ffn, \
             tc.tile_pool(name="htp", bufs=1) as htp, \
             tc.tile_pool(name="ps1", bufs=2, space="PSUM") as ps1p, \
             tc.tile_pool(name="ps2", bufs=2, space="PSUM") as ps2p, \
             tc.tile_pool(name="pst", bufs=2, space="PSUM") as pstp:
            for c in range(NCHUNK):
                xg = ffn.tile([128, NBLK, D], BF16, tag="xg")
                nc.gpsimd.dma_gather(
                    out_ap=xg,
                    in_ap=x_bf16[:, :],
                    idxs_ap=dIdx[:, c * (CB // 16):(c + 1) * (CB // 16)],
                    num_idxs=CB,
                    num_idxs_reg=CB,
                    elem_size=D,
                    transpose=False)
                xTb = ffn.tile([128, DCH, CB], BF16, tag="xTb")
                for dch in range(DCH):
                    psT = pstp.tile([128, CB], BF16, tag="psT")
                    for blk in range(NBLK):
                        nc.tensor.transpose(
                            psT[:, blk * 128:(blk + 1) * 128],
                            xg[:, blk, dch * 128:(dch + 1) * 128],
                            ident_b)
                    nc.vector.tensor_copy(xTb[:, dch, :], psT)
                hT = htp.tile([128, HCH, CB], BF16, tag="hT")
                for j in range(HCH):
                    ps1 = ps1p.tile([128, CB], FP32, tag="ps1")
                    for dch in range(DCH):
                        nc.tensor.matmul(
                            ps1, lhsT=W1s[:, dch, j * 128:(j + 1) * 128],
                            rhs=xTb[:, dch, :],
                            start=(dch == 0), stop=(dch == DCH - 1))
                    nc.scalar.activation(hT[:, j, :], ps1, AF.Silu,
                                         bias=b1s[:, j:j + 1])
                for cs in range(NBLK):
                    osb = ffn.tile([128, D], BF16, tag="osb")
                    blk = c * NBLK + cs
                    for dh in range(2):
                        ps2 = ps2p.tile([128, 512], FP32, tag="ps2")
                        for j in range(HCH):
                            nc.tensor.matmul(
                                ps2,
                                lhsT=hT[:, j, cs * 128:(cs + 1) * 128],
                                rhs=W2s[:, j, dh * 512:(dh + 1) * 512],
                                start=(j == 0), stop=False)
                        nc.tensor.matmul(
                            ps2, lhsT=ones1b,
                            rhs=b2s[:, dh * 512:(dh + 1) * 512],
                            start=False, stop=True)
                        nc.vector.tensor_scalar(
                            osb[:, dh * 512:(dh + 1) * 512], ps2,
                            gdp[:, blk:blk + 1], None, op0=ALU.mult)
                    nc.sync.dma_start(
                        out_e[(c * NBLK + cs) * 128:
                              (c * NBLK + cs + 1) * 128, :],
                        osb)
                if AG_CHUNKED:
                    nc.gpsimd.collective_compute(
                        "AllGather", ALU.bypass,
                        replica_groups=[list(range(NC))],
                        ins=[out_e[c * CB:(c + 1) * CB, :]],
                        outs=[all_out[c]])
            if not AG_CHUNKED:
                nc.gpsimd.collective_compute(
                    "AllGather", ALU.bypass,
                    replica_groups=[list(range(NC))],
                    ins=[out_e.opt()], outs=[all_out.opt()])
            if dbg:
                nc.sync.dma_start(dbg_oe[:, :], out_e)

        # ---------- P4: combine own shard ----------
        with tc.tile_pool(name="comb", bufs=2) as comb:
            if AG_CHUNKED:
                allv = all_out.rearrange("n r c d -> (n r c) d")
            else:
                allv = all_out.rearrange("r c d -> (r c) d")
            GC = 4  # t-tiles per gather (512 idxs)
            for t0 in range(0, TT, GC):
                gk = []
                for k in range(K):
                    gkt = comb.tile([128, GC, D], BF16, tag=f"gk{k}")
                    gk.append(gkt)
                    nc.gpsimd.dma_gather(
                        out_ap=gkt,
                        in_ap=allv,
                        idxs_ap=ci16[:, (k * TT + t0) * 8:
                                     (k * TT + t0 + GC) * 8],
                        num_idxs=GC * 128,
                        num_idxs_reg=GC * 128,
                        elem_size=D,
                        transpose=False)
                for tr in range(GC):
                    t = t0 + tr
                    ysb = comb.tile([128, D], FP32, tag="ysb")
                    nc.vector.tensor_tensor(ysb, gk[0][:, tr, :],
                                            gk[1][:, tr, :], ALU.add)
                    nc.sync.dma_start(y_out[t * 128:(t + 1) * 128, :], ysb)

        top.close()

    nc.compile()
    return nc


# ---------------- host-side staging ----------------

def bfloat16_np():
    import ml_dtypes
    return ml_dtypes.bfloat16


def stage_inputs(cfg: Cfg, x, Wr, br, W1, b1, W2, b2):
    """x: [N, D] fp32; returns list of per-core input dicts."""
    E, D, H, TPC, NC = cfg.E, cfg.D, cfg.H, cfg.TPC, cfg.ncores
    DCH, HCH, TT = cfg.DCH, cfg.HCH, cfg.TT
    x = np.ascontiguousarray(x, np.float32)
    x_bf = x.astype(bfloat16_np())
    ltri = np.tril(np.ones((128, 128), np.float32), -1).astype(bfloat16_np())
    in_maps = []
    for r in range(NC):
        shard = x[r * TPC:(r + 1) * TPC]
        m = {
            "xT_shard": np.ascontiguousarray(shard.T),
            "x_bf16": x_bf,
            "Wr_in": np.ascontiguousarray(
                Wr.reshape(DCH, 128, E).transpose(1, 0, 2)).astype(np.float32),
            "br_in": br.reshape(1, E).astype(np.float32),
            "W1_in": np.ascontiguousarray(
                W1[r].reshape(DCH, 128, H).transpose(1, 0, 2)
            ).astype(bfloat16_np()),
            "W2_in": np.ascontiguousarray(
                W2[r].reshape(HCH, 128, D).transpose(1, 0, 2)
            ).astype(bfloat16_np()),
            "b1_in": np.ascontiguousarray(
                b1[r].reshape(HCH, 128).T).astype(np.float32),
            "b2_in": b2[r].reshape(1, D).astype(np.float32).astype(
                bfloat16_np()),
            "ltri_in": ltri,
            "rk_in": np.full((128, 1), r * TT, np.float32),
        }
        in_maps.append(m)
    return in_maps


# ---------------- problem binding ----------------

import math as _math

B, T = 8, 2048
_N = B * T
_D = 1024
_CAP = int(_math.ceil(1.2 * _N / 8))  # 2458

_CACHE = {}


def _get_nc():
    if "nc" not in _CACHE:
        cfg = Cfg(D=_D, H=4096, TPC=_N // 8, cap=_CAP, CAPP=2560, CB=512)
        _CACHE["cfg"] = cfg
        _CACHE["nc"] = build(cfg, dbg=bool(int(os.environ.get("KDBG", "0"))))
    return _CACHE["cfg"], _CACHE["nc"]


TRACE = False
_LAST_EXEC_NS = None
_LAST_RES = None


def kernel(x_btd, Wr, br, W1, b1, W2, b2):
    from concourse.bass_utils import run_bass_kernel_spmd

    global _LAST_EXEC_NS, _LAST_RES
    cfg, nc = _get_nc()
    x = np.ascontiguousarray(np.asarray(x_btd), np.float32).reshape(_N, _D)
    in_maps = stage_inputs(
        cfg, x, np.asarray(Wr), np.asarray(br), np.asarray(W1),
        np.asarray(b1), np.asarray(W2), np.asarray(b2))
    if TRACE:
        import shutil
        tdir = "/root/problem/work/trace"
        shutil.rmtree(tdir, ignore_errors=True)
        os.makedirs(tdir, exist_ok=True)
        tcores = (list(range(8))
                  if os.environ.get("KTRACE_ALL", "0") == "1" else [0])
        res = run_bass_kernel_spmd(nc, in_maps, list(range(8)), trace=True,
                                   trace_cores=tcores, tmpdir=tdir)
        _LAST_RES = res
        if getattr(res, "exec_time_ns", None):
            _LAST_EXEC_NS = res.exec_time_ns
    else:
        res = run_bass_kernel_spmd(nc, in_maps, list(range(8)))
        _LAST_RES = res
    ys = [res.results[r]["y_out"] for r in range(8)]
    y = np.concatenate(ys, axis=0).astype(np.float32)
    return y.reshape(B, T, _D)
